# revision 39
# baseline (speedup 1.0000x reference)
"""Trainium2 Bass kernel for nn_BasicRecurrentEntityEncoder.

Full-input contract: kernel(**inputs) takes the complete (unsharded) numpy
inputs and returns the full [B, K, D] float32 output. Internally the batch
is sharded over 8 NeuronCores (data parallel, no collectives), the embedding
bag-of-words gather runs through dma_gather against a per-core compacted
bf16 table, and the 64-step entity recurrence runs in a transposed
[D, (b,k)] layout with bf16 matmul operands.

Key device-side structure per core (B_local=16, K=32, D=256, S=64):
  - 8 gather groups of 128 sentences (4096 tokens, 1 dma_gather each);
    word-sum via block-ones matmuls into PSUM; TensorE transpose to build
    E^T [256, 1024] incrementally.
  - precompute  kVT = V^T keys^T  and  eW = W^T E^T  once per group.
  - the scan runs as TWO independent batch groups (b 0-7 | b 8-15), each
    with its own PSUM banks, so their serial dependency chains pipeline
    across engines.
  - per-step chain is latency-optimized: the gate path is issued FIRST
    (psg matmuls before the U matmuls so the ScalarE exp is not stuck
    behind the relu), the sentence mask is folded into the exp's
    per-partition bias (masked row -> +40 -> exp huge -> gate ~ 0, exact
    up to ~2e-6 because h is always 0 or unit-norm), the sigmoid recip
    and the block-diag junk-kill run as ONE custom DVE op (the "+1" of
    the sigmoid denominator is 1e12 on off-diagonal columns), and the
    two d-halves of the sum-of-squares run as ONE custom DVE op feeding
    a single 128-partition reduce matmul.
  - gather-group processing is interleaved into the scan emission
    (group g+1 is digested at step 8g+4) so the shared PSUM-bank tag
    slots alternate gather/scan; the word-sum burst runs half at demoted
    priority (fills PE idle gaps) and half at normal priority (ends the
    inv-bank WAR sooner).
  - the final state is DMAed out transposed; the host untransposes.
  - ScalarE: every function (Exp/Relu/Ln) lives in the
    natural_log_exp_and_others activation table so no table reloads
    occur (the default greedy chooser is patched out).
"""

import sys

if "/opt/trn_rl_repo" not in sys.path:
    sys.path.insert(0, "/opt/trn_rl_repo")

import numpy as np
import ml_dtypes

from concourse import bacc, mybir
import concourse.bass as bass
import concourse.tile as tile
from concourse.bass_utils import run_bass_kernel_spmd
from concourse.masks import make_identity

# Force every ScalarE activation onto the one table set that covers all the
# functions this kernel uses (relu/exp/ln/copy/identity). The default
# chooser greedily picks the first set per function, inserting a ~1283ns
# table reload per pair on the critical path. Padding the dict keeps
# act_func_set_id indices aligned with act_info.json while making only the
# all-covering set usable.
_ONE_SET = "natural_log_exp_and_others"


import concourse.hw_specs as _hw_specs
_ORIG_TABLES = _hw_specs.get_activation_tables


def _patched_tables(module_arch):
    real = _ORIG_TABLES(module_arch)
    names = list(real.keys())
    assert _ONE_SET in names, names
    out = {}
    for n in names:
        if n == _ONE_SET:
            out[n] = real[n]
            break
        out[n] = set()
    return out


def _install_table_patch():
    import functools
    cached = functools.cache(_patched_tables)
    bacc.get_activation_tables = cached
    _hw_specs.get_activation_tables = cached


_install_table_patch()

# ---------------------------------------------------------------------------
# Custom DVE ops.
#
# RECIP1P_BD: out ~= 1/(in0 + in1) in ONE VectorE instruction:
#   u = in0+in1; seed y0 = bitcast(~bits(u)); t = u*y0 lands in [-4.5, -4]
#   for any positive u; quadratic minimax fixup P(t) ~= 1/t gives
#   1/u = y0*P(t) at ~1e-5 relative error. in1 plays the "+1" of the
#   sigmoid on diagonal (own-batch) columns and is 1e12 on off-diagonal
#   columns, so junk logits produce a gate <= 1e-12 and vanish in the
#   row-sum broadcast matmul. The DVE pipeline has exactly 8 ALU blocks;
#   this uses all 8 (a separate mask multiply would not fit).
#
# SQADD: out = in0^2 + in1^2 — fuses the two d-halves of the per-column
#   sum-of-squares so one 128-partition reduce matmul finishes the norm.
# ---------------------------------------------------------------------------
import concourse.dve_ops as _dve_ops
from concourse.dve_spec import AluOp as _AluOp, Bin as _Bin, Spec as _Spec
from concourse.dve_spec import C0 as _C0, C1 as _C1, C2 as _C2, One as _One
from concourse.dve_spec import Src0 as _Src0, Src1 as _Src1, lower as _dve_lower
from concourse.dve_spec import _has_src1 as _dve_has_src1
from concourse.dve_uop import DveOpSpec as _DveOpSpec

_R1P_C2, _R1P_C1, _R1P_C0 = (lambda c: (c[0], c[1], c[2]))(
    np.polyfit(np.linspace(-4.5, -4.0, 2001),
               1.0 / np.linspace(-4.5, -4.0, 2001), 2))


def _recip1p_bd_ref(in0, in1, c0, c1, c2):
    u = (np.asarray(in0, np.float32) + np.asarray(in1, np.float32)).astype(
        np.float32)
    y0 = (~u.view(np.int32)).view(np.float32)
    t = u * y0
    return y0 * (c0 + t * (c1 + c2 * t))


def _sqadd_ref(in0, in1, c0, c1, c2):
    a = np.asarray(in0, np.float32)
    b = np.asarray(in1, np.float32)
    return a * a + b * b


def _register_op(name, spec):
    row = 1 + len(_dve_ops.OPS)
    assert row < 0x20
    shas = {}
    for ver in ("v3", "v4"):
        s = _DveOpSpec(name=name, opcode=row, uops=_dve_lower(spec, ver=ver),
                       rd1_en=_dve_has_src1(spec))
        shas[ver] = s.sha(ver)
    op = _dve_ops.DveOp(name, spec, subdim=False, uops_sha=shas)
    _dve_ops.OPS.append(op)
    _dve_ops._SUB_OPCODE_FOR_NAME[name] = row
    _dve_ops.CUSTOM_DVE_SPECS[name] = spec
    return op


def _make_recip1p_bd():
    u = _Bin(_AluOp.ADD, _Src0, _Src1)
    y0 = _Bin(_AluOp.BITWISE_NOT, u, u)
    t = u * y0
    body = y0 * (_C0 + t * (_C1 + _C2 * t))
    return _register_op("RECIP1P_BD_ANT",
                        _Spec(body=body, reference=_recip1p_bd_ref))


def _make_sqadd():
    body = _Src0 * _Src0 + _Src1 * _Src1
    return _register_op("SQADD_ANT", _Spec(body=body, reference=_sqadd_ref))


_RECIP1P_BD = _make_recip1p_bd()
_SQADD = _make_sqadd()

F32 = mybir.dt.float32
BF16 = mybir.dt.bfloat16
I16 = mybir.dt.int16
AF = mybir.ActivationFunctionType
OP = mybir.AluOpType

B, S, L, K, D = 128, 64, 32, 32, 256
NC = 8
BL = B // NC              # 16 batch rows per core
BK = BL * K               # 512 = free dim of the state
NG = 8                    # gather groups per core (128 sentences each)
TOKG = 128 * L            # 4096 tokens per group
TABLE_ROWS = 32768        # compacted per-core vocab (unique ids <= 32768)
EPS = 1e-12
MASK_BIAS = 40.0          # masked sentence: exp(-logit + 40) -> gate ~ 0

_CACHED = {}


def _build_program():
    nc = bacc.Bacc("TRN2", target_bir_lowering=False, debug=False, num_devices=NC)

    table = nc.dram_tensor("table", [TABLE_ROWS, D], BF16, kind="ExternalInput").ap()
    idx16 = nc.dram_tensor("idx16", [128, NG * TOKG // 16], I16, kind="ExternalInput").ap()
    keysT = nc.dram_tensor("keysT", [D, BK], BF16, kind="ExternalInput").ap()
    Umat = nc.dram_tensor("Umat", [D, D], BF16, kind="ExternalInput").ap()
    Vmat = nc.dram_tensor("Vmat", [D, D], BF16, kind="ExternalInput").ap()
    Wmat = nc.dram_tensor("Wmat", [D, D], BF16, kind="ExternalInput").ap()
    mbias = nc.dram_tensor("mbias", [8, 2 * S], F32, kind="ExternalInput").ap()
    bdin = nc.dram_tensor("bdin", [8, 256], BF16, kind="ExternalInput").ap()
    hout = nc.dram_tensor("hout", [2 * 128, BK], BF16, kind="ExternalOutput").ap()

    with tile.TileContext(nc) as tc:
        _emit(nc, tc, table, idx16, keysT, Umat, Vmat, Wmat, mbias, bdin, hout)
    nc.compile()
    return nc


def _emit(nc, tc, table, idx16, keysT, Umat, Vmat, Wmat, mbias, bdin, hout):
    from contextlib import ExitStack

    ctx = ExitStack()
    const = ctx.enter_context(tc.tile_pool(name="const", bufs=1))
    persist = ctx.enter_context(tc.tile_pool(name="persist", bufs=1))
    gpool = ctx.enter_context(tc.tile_pool(name="g", bufs=2))
    work = ctx.enter_context(tc.tile_pool(name="work", bufs=4))
    hpool = ctx.enter_context(tc.tile_pool(name="h", bufs=3))
    # PSUM budget: 8 banks. psh{0,1} = 2; psbg{0,1} (gate broadcast) = 2;
    # pnv{0,1} (inv broadcast) = 2; pm{0,1} (psg logits + pss sumsq; also
    # reused by the gather phase) = 2. The inv broadcast must NOT share the
    # pm bank: the next step's gate matmuls would inherit a WAR dependency
    # on hn's read of it, putting them back on the critical chain.
    psH = ctx.enter_context(tc.tile_pool(name="psH", bufs=1, space="PSUM"))
    psS = ctx.enter_context(tc.tile_pool(name="psS", bufs=1, space="PSUM"))

    # ---- constants into SBUF ----
    sb_idx = const.tile([128, NG * TOKG // 16], I16)
    nc.sync.dma_start(out=sb_idx[:, 0:TOKG // 16], in_=idx16[:, 0:TOKG // 16])
    nc.sync.dma_start(out=sb_idx[:, TOKG // 16:], in_=idx16[:, TOKG // 16:])
    kT = [const.tile([128, BK], BF16, tag=f"kT{j}", name=f"kT{j}") for j in range(2)]
    for j in range(2):
        nc.sync.dma_start(out=kT[j][:], in_=keysT[128 * j:128 * (j + 1), :])
    sbU = [const.tile([128, D], BF16, tag=f"sbU{j}", name=f"sbU{j}") for j in range(2)]
    sbV = [const.tile([128, D], BF16, tag=f"sbV{j}", name=f"sbV{j}") for j in range(2)]
    sbW = [const.tile([128, D], BF16, tag=f"sbW{j}", name=f"sbW{j}") for j in range(2)]
    for j in range(2):
        nc.sync.dma_start(out=sbU[j][:], in_=Umat[128 * j:128 * (j + 1), :])
        nc.sync.dma_start(out=sbV[j][:], in_=Vmat[128 * j:128 * (j + 1), :])
        nc.sync.dma_start(out=sbW[j][:], in_=Wmat[128 * j:128 * (j + 1), :])
    sb_mb = const.tile([8, 2 * S], F32)
    nc.sync.dma_start(out=sb_mb[:], in_=mbias[:])

    I128 = const.tile([128, 128], BF16)
    make_identity(nc, I128[:])
    ones8 = const.tile([8, 128], BF16)
    nc.vector.memset(ones8[:], 1.0)
    ones128 = const.tile([128, 1], BF16)
    nc.vector.memset(ones128[:], 1.0)
    ones1 = const.tile([1, 128], BF16)
    nc.vector.memset(ones1[:], 1.0)
    epsap = const.tile([1, 1], F32)
    nc.vector.memset(epsap[:], EPS)
    # sigmoid denominator offset: 1.0 on own-batch (diagonal) columns, 1e12
    # elsewhere so junk logits produce a ~0 gate (see RECIP1P_BD above).
    bd8 = const.tile([8, 256], BF16)
    nc.sync.dma_start(out=bd8[:], in_=bdin[:])
    # word-sum reducers: Ablk[i][p, m] = 1 iff m == 4*i + p//32.
    Ablk = []
    for i in range(16):
        a = const.tile([128, 64], BF16, tag=f"Ablk{i}", name=f"Ablk{i}")
        nc.vector.memset(a[:], 0.0)
        for q in range(4):
            nc.vector.memset(a[32 * q:32 * (q + 1), 4 * i + q:4 * i + q + 1], 1.0)
        Ablk.append(a)

    # ---- persistent intermediates ----
    ET = [persist.tile([128, NG * 128], BF16, tag=f"ET{j}", name=f"ET{j}") for j in range(2)]   # E^T  [d, (g,ds,b)]
    eW = [persist.tile([128, NG * 128], BF16, tag=f"eWt{j}", name=f"eWt{j}") for j in range(2)]   # W^T E^T
    kVT = [persist.tile([128, BK], BF16, tag=f"kVT{j}", name=f"kVT{j}") for j in range(2)]        # V^T keys^T

    # kVT = V^T @ keysT   (out[de, bk] = sum_d V[d,de] keysT[d,bk])
    for m in range(2):
        ps = psH.tile([128, BK], F32, tag="psh1", name="pskv")
        nc.tensor.matmul(ps[:], lhsT=sbV[0][:, 128 * m:128 * (m + 1)], rhs=kT[0][:],
                         start=True, stop=False)
        nc.tensor.matmul(ps[:], lhsT=sbV[1][:, 128 * m:128 * (m + 1)], rhs=kT[1][:],
                         start=False, stop=True)
        nc.vector.tensor_copy(out=kVT[m][:], in_=ps[:])

    # ---- gather machinery ----
    # Group processing is interleaved into the scan emission (group g+1 is
    # digested at step 8g+4) so the shared PSUM-bank tag slots alternate
    # gather/scan naturally; emitting all gathers up front would serialize
    # scan step 0 behind every group's word-sum through the shared banks.
    def _issue_gather(g):
        G = gpool.tile([128, L, D], BF16, tag="G", name=f"G{g}")
        nc.gpsimd.dma_gather(
            out_ap=G[:], in_ap=table[:],
            idxs_ap=sb_idx[:, (TOKG // 16) * g:(TOKG // 16) * (g + 1)],
            num_idxs=TOKG, num_idxs_reg=TOKG, elem_size=D, single_packet=False,
        )
        return G

    def _sum_group(g, G):
        # word-sum: slot c holds words of sentences 4c..4c+3; accumulate
        # 8 slots per 32-aligned PSUM block. psE shares the pnv0 scan bank
        # (longest per-step idle window); the burst runs at demoted
        # priority so simultaneously-ready scan ops win the engine heaps.
        psE = psS.tile([128, D], F32, tag="pnv0", name="psE")
        enc = work.tile([128, D], BF16, tag="enc", name=f"enc{g}")
        with tc.high_priority(offset=-10**6):
            for c in range(L // 2):
                j, i = c // 16, c % 16
                nc.tensor.matmul(psE[64 * j:64 * (j + 1), :], lhsT=Ablk[i][:],
                                 rhs=G[:, c, :], start=(i == 0), stop=(i == 15))
        # second half at normal priority: ends the burst (and the bank WAR)
        # sooner at the cost of briefly preempting scan matmuls
        for c in range(L // 2, L):
            j, i = c // 16, c % 16
            nc.tensor.matmul(psE[64 * j:64 * (j + 1), :], lhsT=Ablk[i][:],
                             rhs=G[:, c, :], start=(i == 0), stop=(i == 15))
        nc.scalar.copy(out=enc[:], in_=psE[:])
        return enc

    def _expand_group(g, enc):
        with tc.high_priority(offset=-10**6):
            # transpose -> ET columns for this group
            for j in range(2):
                pt = psS.tile([128, 128], BF16, tag="pm1", name="pt")
                nc.tensor.transpose(pt[:], enc[:, 128 * j:128 * (j + 1)], I128[:])
                nc.vector.tensor_copy(out=ET[j][:, 128 * g:128 * (g + 1)],
                                      in_=pt[:])
            # eW = W^T @ ET_g
            for m in range(2):
                pw = psS.tile([128, 128], F32, tag="pm1", name="pw")
                nc.tensor.matmul(pw[:], lhsT=sbW[0][:, 128 * m:128 * (m + 1)],
                                 rhs=ET[0][:, 128 * g:128 * (g + 1)],
                                 start=True, stop=False)
                nc.tensor.matmul(pw[:], lhsT=sbW[1][:, 128 * m:128 * (m + 1)],
                                 rhs=ET[1][:, 128 * g:128 * (g + 1)],
                                 start=False, stop=True)
                nc.vector.tensor_copy(out=eW[m][:, 128 * g:128 * (g + 1)],
                                      in_=pw[:])

    gtiles = {}
    gencs = {}
    gtiles[0] = _issue_gather(0)
    gtiles[1] = _issue_gather(1)
    _expand_group(0, _sum_group(0, gtiles.pop(0)))

    # ---- scan: two independent batch groups (b 0-7 | b 8-15) pipelined ----
    # State h[gb] [128, 512]: partitions = d mod 128, columns pack
    # (d-half, bk) for the group's 256 bk entries.
    HB = BK // 2  # 256
    h = [hpool.tile([128, BK], BF16, tag=f"h{gb}", name=f"h{gb}")
         for gb in range(2)]
    for gb in range(2):
        nc.vector.memset(h[gb][:], 0.0)

    for t in range(S):
        g, ds = t // 8, t % 8
        if ds == 4 and g + 1 < NG:
            if g + 2 < NG:
                gtiles[g + 2] = _issue_gather(g + 2)
            _expand_group(g + 1, _sum_group(g + 1, gtiles.pop(g + 1)))
        hn = [None, None]
        for gb in range(2):
            cg = 128 * g + 16 * ds + 8 * gb  # ET/eW cols for this step+group
            bks = slice(HB * gb, HB * (gb + 1))
            hg = h[gb]

            # --- gate path first: its ScalarE exp must not queue behind
            # the relu, and its PE matmuls must finish before the U ones.
            pm = psS.tile([128, BK], F32, tag=f"pm{gb}", name=f"pm{gb}")
            psg = pm[0:8, 0:HB]
            nc.tensor.matmul(psg, lhsT=ET[0][:, cg:cg + 8], rhs=kT[0][:, bks],
                             start=True, stop=False)
            nc.tensor.matmul(psg, lhsT=ET[1][:, cg:cg + 8], rhs=kT[1][:, bks],
                             start=False, stop=False)
            nc.tensor.matmul(psg, lhsT=ET[0][:, cg:cg + 8], rhs=hg[:, 0:HB],
                             start=False, stop=False)
            nc.tensor.matmul(psg, lhsT=ET[1][:, cg:cg + 8], rhs=hg[:, HB:BK],
                             start=False, stop=True)
            # eg = exp(-logits + mask_bias); masked rows get +45 so the
            # sigmoid underflows to 0 and h carries through unchanged.
            eg = work.tile([8, HB], F32, tag=f"eg{gb}", name=f"eg{gb}")
            gm = work.tile([8, HB], BF16, tag=f"gm{gb}", name=f"gm{gb}")
            nc.scalar.activation(eg[:], psg, AF.Exp, scale=-1.0,
                                 bias=sb_mb[0:8, 2 * t + gb:2 * t + gb + 1])
            # gate = 1/(eg + bd): bd=1 on-diag, 1e12 off-diag -> gate ~ 0
            nc.vector._custom_dve(_RECIP1P_BD, out=gm[:], in0=eg[:],
                                  in1=bd8[:],
                                  s0=float(_R1P_C0), s1=float(_R1P_C1),
                                  imm2=float(_R1P_C2))

            # --- h_tilda pre-activation: kVT + eW + U^T h (h terms last)
            pshG = psH.tile([128, BK], F32, tag=f"psh{gb}", name=f"psh{gb}")
            for m in range(2):
                msl = slice(HB * m, HB * (m + 1))
                nc.tensor.matmul(pshG[:, msl], lhsT=I128[:], rhs=kVT[m][:, bks],
                                 start=True, stop=False)
                ew_bc = eW[m][:, cg:cg + 8].unsqueeze(2).broadcast_to([128, 8, 32])
                nc.tensor.matmul(pshG[:, msl], lhsT=I128[:], rhs=ew_bc,
                                 start=False, stop=False)
                nc.tensor.matmul(pshG[:, msl], lhsT=sbU[0][:, 128 * m:128 * (m + 1)],
                                 rhs=hg[:, 0:HB], start=False, stop=False)
                nc.tensor.matmul(pshG[:, msl], lhsT=sbU[1][:, 128 * m:128 * (m + 1)],
                                 rhs=hg[:, HB:BK], start=False, stop=True)

            # gate broadcast (both column halves)
            psbg = psS.tile([128, BK], F32, tag=f"psbg{gb}", name=f"psbg{gb}")
            nc.tensor.matmul(psbg[:, 0:HB], lhsT=ones8[:], rhs=gm[:],
                             start=True, stop=True)
            nc.tensor.matmul(psbg[:, HB:BK], lhsT=ones8[:], rhs=gm[:],
                             start=True, stop=True)

            # r = relu(psh); u = r*gate; upd = u + h
            r = work.tile([128, BK], BF16, tag=f"r{gb}", name=f"r{gb}")
            nc.scalar.activation(r[:], pshG[:], AF.Relu)
            u = work.tile([128, BK], BF16, tag=f"u{gb}", name=f"u{gb}")
            nc.vector.tensor_tensor(out=u[:], in0=r[:], in1=psbg[:, 0:BK], op=OP.mult)
            upd = work.tile([128, BK], BF16, tag=f"upd{gb}", name=f"upd{gb}")
            nc.vector.tensor_tensor(out=upd[:], in0=u[:], in1=hg[:], op=OP.add)

            # sumsq: fused d-half square-add then one 128-partition reduce
            sqh = work.tile([128, HB], BF16, tag=f"sq{gb}", name=f"sq{gb}")
            nc.vector._custom_dve(_SQADD, out=sqh[:], in0=upd[:, 0:HB],
                                  in1=upd[:, HB:BK])
            pss = pm[0:1, HB:BK]
            lns = work.tile([1, HB], F32, tag=f"lns{gb}", name=f"lns{gb}")
            inv = work.tile([1, HB], BF16, tag=f"inv{gb}", name=f"inv{gb}")
            pnv = psS.tile([128, BK], F32, tag=f"pnv{gb}", name=f"pnv{gb}")
            hn[gb] = hpool.tile([128, BK], BF16, tag=f"h{gb}", name=f"hn{gb}")
            nc.tensor.matmul(pss, lhsT=ones128[:], rhs=sqh[:],
                             start=True, stop=True)
            nc.scalar.activation(lns[:], pss, AF.Ln, bias=epsap[:])
            nc.scalar.activation(inv[:], lns[:], AF.Exp, scale=-0.5)
            nc.tensor.matmul(pnv[:, 0:HB], lhsT=ones1[:], rhs=inv[:],
                             start=True, stop=True)
            nc.tensor.matmul(pnv[:, HB:BK], lhsT=ones1[:], rhs=inv[:],
                             start=True, stop=True)
            nc.vector.tensor_tensor(out=hn[gb][:], in0=upd[:],
                                    in1=pnv[:, 0:BK], op=OP.mult)
        h = hn

    # ---- output: dump the transposed state; the host untransposes ----
    for gb in range(2):
        nc.sync.dma_start(out=hout[128 * gb:128 * (gb + 1), :], in_=h[gb][:])

    ctx.close()


def _prep_core(pr, mask, keys_c, emb):
    """Host-side marshaling for one core's shard."""
    uniq, inv = np.unique(pr, return_inverse=True)
    assert len(uniq) <= TABLE_ROWS
    table = np.zeros((TABLE_ROWS, D), dtype=ml_dtypes.bfloat16)
    table[: len(uniq)] = emb[uniq].astype(ml_dtypes.bfloat16)
    ranks = inv.reshape(BL, S, L).astype(np.int16)

    # token order per group g: i = (ds*16 + b)*32 + w
    idx_groups = []
    for g in range(NG):
        blk = ranks[:, 8 * g:8 * (g + 1), :]          # [b, ds, w]
        lst = blk.transpose(1, 0, 2).reshape(-1)      # [(ds, b, w)] length 4096
        idx_groups.append(np.tile(lst.reshape(TOKG // 16, 16).T, (8, 1)))
    idx16 = np.concatenate(idx_groups, axis=1).astype(np.int16)  # [128, NG*256]

    keysT = np.ascontiguousarray(
        keys_c.reshape(BK, D).T).astype(ml_dtypes.bfloat16)      # [256, 512]
    # mb[j, 2t+gb] = MASK_BIAS * (1 - mask[8*gb + j, t])
    m = mask.astype(np.float32)                                  # [16, 64]
    mb = np.zeros((8, 2 * S), np.float32)
    for gb in range(2):
        mb[:, gb::2] = MASK_BIAS * (1.0 - m[8 * gb:8 * (gb + 1), :])
    return table, idx16, keysT, mb


def kernel(prgrph, prgrph_mask, keys, embedding_matrix, U, V, W):
    prgrph = np.asarray(prgrph)
    prgrph_mask = np.asarray(prgrph_mask)
    keys = np.asarray(keys, dtype=np.float32)
    emb = np.asarray(embedding_matrix, dtype=np.float32)
    U = np.asarray(U, dtype=np.float32)
    V = np.asarray(V, dtype=np.float32)
    W = np.asarray(W, dtype=np.float32)

    if "nc" not in _CACHED:
        _CACHED["nc"] = _build_program()
    nc = _CACHED["nc"]

    Ub, Vb, Wb = (x.astype(ml_dtypes.bfloat16) for x in (U, V, W))
    bdc = np.where(np.arange(8)[:, None] == (np.arange(256)[None, :] // K),
                   1.0, 1e12).astype(ml_dtypes.bfloat16)

    in_maps = []
    for c in range(NC):
        sl = slice(BL * c, BL * (c + 1))
        table, idx16, keysT, mb = _prep_core(
            prgrph[sl], prgrph_mask[sl, :, 0], keys[sl], emb)
        in_maps.append({
            "table": table, "idx16": idx16, "keysT": keysT,
            "Umat": Ub, "Vmat": Vb, "Wmat": Wb,
            "mbias": mb, "bdin": bdc,
        })

    res = run_bass_kernel_spmd(nc, in_maps, core_ids=list(range(NC)))
    outs = []
    for c in range(NC):
        ht = np.asarray(res.results[c]["hout"], dtype=np.float32)  # [256, 512]
        # rows: [gb*128 + p] = d half m at partition p; cols: gb-local bk.
        # h^T[d, bk_global]: d = m*128 + p, bk_global = gb*256 + bk_local
        full = np.zeros((D, BK), np.float32)
        for gb in range(2):
            blk = ht[128 * gb:128 * (gb + 1), :]      # [128, 512] = (p, (m, bk))
            full[0:128, 256 * gb:256 * (gb + 1)] = blk[:, 0:256]
            full[128:256, 256 * gb:256 * (gb + 1)] = blk[:, 256:512]
        outs.append(full.T.reshape(BL, K, D))
    out = np.concatenate(outs, axis=0)
    return out.astype(np.float32)


# revision 45
# speedup vs baseline: 1.0002x; 1.0002x over previous
"""Trainium2 Bass kernel for nn_BasicRecurrentEntityEncoder.

Full-input contract: kernel(**inputs) takes the complete (unsharded) numpy
inputs and returns the full [B, K, D] float32 output. Internally the batch
is sharded over 8 NeuronCores (data parallel, no collectives), the embedding
bag-of-words gather runs through dma_gather against a per-core compacted
bf16 table, and the 64-step entity recurrence runs in a transposed
[D, (b,k)] layout with bf16 matmul operands.

Key device-side structure per core (B_local=16, K=32, D=256, S=64):
  - 8 gather groups of 128 sentences (4096 tokens, 1 dma_gather each);
    word-sum via block-ones matmuls into PSUM; TensorE transpose to build
    E^T [256, 1024] incrementally.
  - precompute  kVT = V^T keys^T  and  eW = W^T E^T  once per group.
  - the scan runs as TWO independent batch groups (b 0-7 | b 8-15), each
    with its own PSUM banks, so their serial dependency chains pipeline
    across engines.
  - per-step chain is latency-optimized: the gate path is issued FIRST
    (psg matmuls before the U matmuls so the ScalarE exp is not stuck
    behind the relu), the sentence mask is folded into the exp's
    per-partition bias (masked row -> +40 -> exp huge -> gate ~ 0, exact
    up to ~2e-6 because h is always 0 or unit-norm), the sigmoid recip
    and the block-diag junk-kill run as ONE custom DVE op (the "+1" of
    the sigmoid denominator is 1e12 on off-diagonal columns), and the
    two d-halves of the sum-of-squares run as ONE custom DVE op feeding
    a single 128-partition reduce matmul.
  - gather-group processing is interleaved into the scan emission
    (group g+1 is digested at step 8g+5) so the shared PSUM-bank tag
    slots alternate gather/scan; the word-sum burst runs half at demoted
    priority (fills PE idle gaps) and half at normal priority (ends the
    inv-bank WAR sooner).
  - the final state is DMAed out transposed; the host untransposes.
  - ScalarE: every function (Exp/Relu/Ln) lives in the
    natural_log_exp_and_others activation table so no table reloads
    occur (the default greedy chooser is patched out).
"""

import sys

if "/opt/trn_rl_repo" not in sys.path:
    sys.path.insert(0, "/opt/trn_rl_repo")

import numpy as np
import ml_dtypes

from concourse import bacc, mybir
import concourse.bass as bass
import concourse.tile as tile
from concourse.bass_utils import run_bass_kernel_spmd
from concourse.masks import make_identity

# Force every ScalarE activation onto the one table set that covers all the
# functions this kernel uses (relu/exp/ln/copy/identity). The default
# chooser greedily picks the first set per function, inserting a ~1283ns
# table reload per pair on the critical path. Padding the dict keeps
# act_func_set_id indices aligned with act_info.json while making only the
# all-covering set usable.
_ONE_SET = "natural_log_exp_and_others"


import concourse.hw_specs as _hw_specs
_ORIG_TABLES = _hw_specs.get_activation_tables


def _patched_tables(module_arch):
    real = _ORIG_TABLES(module_arch)
    names = list(real.keys())
    assert _ONE_SET in names, names
    out = {}
    for n in names:
        if n == _ONE_SET:
            out[n] = real[n]
            break
        out[n] = set()
    return out


def _install_table_patch():
    import functools
    cached = functools.cache(_patched_tables)
    bacc.get_activation_tables = cached
    _hw_specs.get_activation_tables = cached


_install_table_patch()

# ---------------------------------------------------------------------------
# Custom DVE ops.
#
# RECIP1P_BD: out ~= 1/(in0 + in1) in ONE VectorE instruction:
#   u = in0+in1; seed y0 = bitcast(~bits(u)); t = u*y0 lands in [-4.5, -4]
#   for any positive u; quadratic minimax fixup P(t) ~= 1/t gives
#   1/u = y0*P(t) at ~1e-5 relative error. in1 plays the "+1" of the
#   sigmoid on diagonal (own-batch) columns and is 1e12 on off-diagonal
#   columns, so junk logits produce a gate <= 1e-12 and vanish in the
#   row-sum broadcast matmul. The DVE pipeline has exactly 8 ALU blocks;
#   this uses all 8 (a separate mask multiply would not fit).
#
# SQADD: out = in0^2 + in1^2 — fuses the two d-halves of the per-column
#   sum-of-squares so one 128-partition reduce matmul finishes the norm.
# ---------------------------------------------------------------------------
import concourse.dve_ops as _dve_ops
from concourse.dve_spec import AluOp as _AluOp, Bin as _Bin, Spec as _Spec
from concourse.dve_spec import C0 as _C0, C1 as _C1, C2 as _C2, One as _One
from concourse.dve_spec import Src0 as _Src0, Src1 as _Src1, lower as _dve_lower
from concourse.dve_spec import _has_src1 as _dve_has_src1
from concourse.dve_uop import DveOpSpec as _DveOpSpec

_R1P_C2, _R1P_C1, _R1P_C0 = (lambda c: (c[0], c[1], c[2]))(
    np.polyfit(np.linspace(-4.5, -4.0, 2001),
               1.0 / np.linspace(-4.5, -4.0, 2001), 2))


def _recip1p_bd_ref(in0, in1, c0, c1, c2):
    u = (np.asarray(in0, np.float32) + np.asarray(in1, np.float32)).astype(
        np.float32)
    y0 = (~u.view(np.int32)).view(np.float32)
    t = u * y0
    return y0 * (c0 + t * (c1 + c2 * t))


def _sqadd_ref(in0, in1, c0, c1, c2):
    a = np.asarray(in0, np.float32)
    b = np.asarray(in1, np.float32)
    return a * a + b * b


def _register_op(name, spec):
    row = 1 + len(_dve_ops.OPS)
    assert row < 0x20
    shas = {}
    for ver in ("v3", "v4"):
        s = _DveOpSpec(name=name, opcode=row, uops=_dve_lower(spec, ver=ver),
                       rd1_en=_dve_has_src1(spec))
        shas[ver] = s.sha(ver)
    op = _dve_ops.DveOp(name, spec, subdim=False, uops_sha=shas)
    _dve_ops.OPS.append(op)
    _dve_ops._SUB_OPCODE_FOR_NAME[name] = row
    _dve_ops.CUSTOM_DVE_SPECS[name] = spec
    return op


def _make_recip1p_bd():
    u = _Bin(_AluOp.ADD, _Src0, _Src1)
    y0 = _Bin(_AluOp.BITWISE_NOT, u, u)
    t = u * y0
    body = y0 * (_C0 + t * (_C1 + _C2 * t))
    return _register_op("RECIP1P_BD_ANT",
                        _Spec(body=body, reference=_recip1p_bd_ref))


def _make_sqadd():
    body = _Src0 * _Src0 + _Src1 * _Src1
    return _register_op("SQADD_ANT", _Spec(body=body, reference=_sqadd_ref))


_RECIP1P_BD = _make_recip1p_bd()
_SQADD = _make_sqadd()

F32 = mybir.dt.float32
BF16 = mybir.dt.bfloat16
I16 = mybir.dt.int16
AF = mybir.ActivationFunctionType
OP = mybir.AluOpType

B, S, L, K, D = 128, 64, 32, 32, 256
NC = 8
BL = B // NC              # 16 batch rows per core
BK = BL * K               # 512 = free dim of the state
NG = 8                    # gather groups per core (128 sentences each)
TOKG = 128 * L            # 4096 tokens per group
TABLE_ROWS = 32768        # compacted per-core vocab (unique ids <= 32768)
EPS = 1e-12
MASK_BIAS = 40.0          # masked sentence: exp(-logit + 40) -> gate ~ 0

_CACHED = {}


def _build_program():
    nc = bacc.Bacc("TRN2", target_bir_lowering=False, debug=False, num_devices=NC)

    table = nc.dram_tensor("table", [TABLE_ROWS, D], BF16, kind="ExternalInput").ap()
    idx16 = nc.dram_tensor("idx16", [128, NG * TOKG // 16], I16, kind="ExternalInput").ap()
    keysT = nc.dram_tensor("keysT", [D, BK], BF16, kind="ExternalInput").ap()
    Umat = nc.dram_tensor("Umat", [D, D], BF16, kind="ExternalInput").ap()
    Vmat = nc.dram_tensor("Vmat", [D, D], BF16, kind="ExternalInput").ap()
    Wmat = nc.dram_tensor("Wmat", [D, D], BF16, kind="ExternalInput").ap()
    mbias = nc.dram_tensor("mbias", [8, 2 * S], F32, kind="ExternalInput").ap()
    bdin = nc.dram_tensor("bdin", [8, 256], BF16, kind="ExternalInput").ap()
    hout = nc.dram_tensor("hout", [2 * 128, BK], BF16, kind="ExternalOutput").ap()

    with tile.TileContext(nc) as tc:
        _emit(nc, tc, table, idx16, keysT, Umat, Vmat, Wmat, mbias, bdin, hout)
    nc.compile()
    return nc


def _emit(nc, tc, table, idx16, keysT, Umat, Vmat, Wmat, mbias, bdin, hout):
    from contextlib import ExitStack

    ctx = ExitStack()
    const = ctx.enter_context(tc.tile_pool(name="const", bufs=1))
    persist = ctx.enter_context(tc.tile_pool(name="persist", bufs=1))
    gpool = ctx.enter_context(tc.tile_pool(name="g", bufs=2))
    work = ctx.enter_context(tc.tile_pool(name="work", bufs=4))
    hpool = ctx.enter_context(tc.tile_pool(name="h", bufs=3))
    # PSUM budget: 8 banks. psh{0,1} = 2; psbg{0,1} (gate broadcast) = 2;
    # pnv{0,1} (inv broadcast) = 2; pm{0,1} (psg logits + pss sumsq; also
    # reused by the gather phase) = 2. The inv broadcast must NOT share the
    # pm bank: the next step's gate matmuls would inherit a WAR dependency
    # on hn's read of it, putting them back on the critical chain.
    psH = ctx.enter_context(tc.tile_pool(name="psH", bufs=1, space="PSUM"))
    psS = ctx.enter_context(tc.tile_pool(name="psS", bufs=1, space="PSUM"))

    # ---- constants into SBUF ----
    sb_idx = const.tile([128, NG * TOKG // 16], I16)
    nc.sync.dma_start(out=sb_idx[:, 0:TOKG // 16], in_=idx16[:, 0:TOKG // 16])
    nc.sync.dma_start(out=sb_idx[:, TOKG // 16:], in_=idx16[:, TOKG // 16:])
    kT = [const.tile([128, BK], BF16, tag=f"kT{j}", name=f"kT{j}") for j in range(2)]
    for j in range(2):
        nc.sync.dma_start(out=kT[j][:], in_=keysT[128 * j:128 * (j + 1), :])
    sbU = [const.tile([128, D], BF16, tag=f"sbU{j}", name=f"sbU{j}") for j in range(2)]
    sbV = [const.tile([128, D], BF16, tag=f"sbV{j}", name=f"sbV{j}") for j in range(2)]
    sbW = [const.tile([128, D], BF16, tag=f"sbW{j}", name=f"sbW{j}") for j in range(2)]
    for j in range(2):
        nc.sync.dma_start(out=sbU[j][:], in_=Umat[128 * j:128 * (j + 1), :])
        nc.sync.dma_start(out=sbV[j][:], in_=Vmat[128 * j:128 * (j + 1), :])
        nc.sync.dma_start(out=sbW[j][:], in_=Wmat[128 * j:128 * (j + 1), :])
    sb_mb = const.tile([8, 2 * S], F32)
    nc.sync.dma_start(out=sb_mb[:], in_=mbias[:])

    I128 = const.tile([128, 128], BF16)
    make_identity(nc, I128[:])
    ones8 = const.tile([8, 128], BF16)
    nc.vector.memset(ones8[:], 1.0)
    ones128 = const.tile([128, 1], BF16)
    nc.vector.memset(ones128[:], 1.0)
    ones1 = const.tile([1, 128], BF16)
    nc.vector.memset(ones1[:], 1.0)
    epsap = const.tile([1, 1], F32)
    nc.vector.memset(epsap[:], EPS)
    # sigmoid denominator offset: 1.0 on own-batch (diagonal) columns, 1e12
    # elsewhere so junk logits produce a ~0 gate (see RECIP1P_BD above).
    bd8 = const.tile([8, 256], BF16)
    nc.sync.dma_start(out=bd8[:], in_=bdin[:])
    # word-sum reducers: Ablk[i][p, m] = 1 iff m == 4*i + p//32.
    Ablk = []
    for i in range(16):
        a = const.tile([128, 64], BF16, tag=f"Ablk{i}", name=f"Ablk{i}")
        nc.vector.memset(a[:], 0.0)
        for q in range(4):
            nc.vector.memset(a[32 * q:32 * (q + 1), 4 * i + q:4 * i + q + 1], 1.0)
        Ablk.append(a)

    # ---- persistent intermediates ----
    ET = [persist.tile([128, NG * 128], BF16, tag=f"ET{j}", name=f"ET{j}") for j in range(2)]   # E^T  [d, (g,ds,b)]
    eW = [persist.tile([128, NG * 128], BF16, tag=f"eWt{j}", name=f"eWt{j}") for j in range(2)]   # W^T E^T
    kVT = [persist.tile([128, BK], BF16, tag=f"kVT{j}", name=f"kVT{j}") for j in range(2)]        # V^T keys^T

    # kVT = V^T @ keysT   (out[de, bk] = sum_d V[d,de] keysT[d,bk])
    for m in range(2):
        ps = psH.tile([128, BK], F32, tag="psh1", name="pskv")
        nc.tensor.matmul(ps[:], lhsT=sbV[0][:, 128 * m:128 * (m + 1)], rhs=kT[0][:],
                         start=True, stop=False)
        nc.tensor.matmul(ps[:], lhsT=sbV[1][:, 128 * m:128 * (m + 1)], rhs=kT[1][:],
                         start=False, stop=True)
        nc.vector.tensor_copy(out=kVT[m][:], in_=ps[:])

    # ---- gather machinery ----
    # Group processing is interleaved into the scan emission (group g+1 is
    # digested at step 8g+4) so the shared PSUM-bank tag slots alternate
    # gather/scan naturally; emitting all gathers up front would serialize
    # scan step 0 behind every group's word-sum through the shared banks.
    def _issue_gather(g):
        G = gpool.tile([128, L, D], BF16, tag="G", name=f"G{g}")
        nc.gpsimd.dma_gather(
            out_ap=G[:], in_ap=table[:],
            idxs_ap=sb_idx[:, (TOKG // 16) * g:(TOKG // 16) * (g + 1)],
            num_idxs=TOKG, num_idxs_reg=TOKG, elem_size=D, single_packet=False,
        )
        return G

    def _sum_group(g, G):
        # word-sum: slot c holds words of sentences 4c..4c+3; accumulate
        # 8 slots per 32-aligned PSUM block. psE shares the pnv0 scan bank
        # (longest per-step idle window); the burst runs at demoted
        # priority so simultaneously-ready scan ops win the engine heaps.
        psE = psS.tile([128, D], F32, tag="pnv0", name="psE")
        enc = work.tile([128, D], BF16, tag="enc", name=f"enc{g}")
        with tc.high_priority(offset=-10**6):
            for c in range(L // 2):
                j, i = c // 16, c % 16
                nc.tensor.matmul(psE[64 * j:64 * (j + 1), :], lhsT=Ablk[i][:],
                                 rhs=G[:, c, :], start=(i == 0), stop=(i == 15))
        # second half at normal priority: ends the burst (and the bank WAR)
        # sooner at the cost of briefly preempting scan matmuls
        for c in range(L // 2, L):
            j, i = c // 16, c % 16
            nc.tensor.matmul(psE[64 * j:64 * (j + 1), :], lhsT=Ablk[i][:],
                             rhs=G[:, c, :], start=(i == 0), stop=(i == 15))
        nc.scalar.copy(out=enc[:], in_=psE[:])
        return enc

    def _expand_group(g, enc):
        with tc.high_priority(offset=-10**6):
            # transpose -> ET columns for this group
            for j in range(2):
                pt = psS.tile([128, 128], BF16, tag="pm1", name="pt")
                nc.tensor.transpose(pt[:], enc[:, 128 * j:128 * (j + 1)], I128[:])
                nc.vector.tensor_copy(out=ET[j][:, 128 * g:128 * (g + 1)],
                                      in_=pt[:])
            # eW = W^T @ ET_g
            for m in range(2):
                pw = psS.tile([128, 128], F32, tag="pm1", name="pw")
                nc.tensor.matmul(pw[:], lhsT=sbW[0][:, 128 * m:128 * (m + 1)],
                                 rhs=ET[0][:, 128 * g:128 * (g + 1)],
                                 start=True, stop=False)
                nc.tensor.matmul(pw[:], lhsT=sbW[1][:, 128 * m:128 * (m + 1)],
                                 rhs=ET[1][:, 128 * g:128 * (g + 1)],
                                 start=False, stop=True)
                nc.vector.tensor_copy(out=eW[m][:, 128 * g:128 * (g + 1)],
                                      in_=pw[:])

    gtiles = {}
    gencs = {}
    gtiles[0] = _issue_gather(0)
    gtiles[1] = _issue_gather(1)
    _expand_group(0, _sum_group(0, gtiles.pop(0)))

    # ---- scan: two independent batch groups (b 0-7 | b 8-15) pipelined ----
    # State h[gb] [128, 512]: partitions = d mod 128, columns pack
    # (d-half, bk) for the group's 256 bk entries.
    HB = BK // 2  # 256
    h = [hpool.tile([128, BK], BF16, tag=f"h{gb}", name=f"h{gb}")
         for gb in range(2)]
    for gb in range(2):
        nc.vector.memset(h[gb][:], 0.0)

    for t in range(S):
        g, ds = t // 8, t % 8
        if ds == 5 and g + 1 < NG:
            if g + 2 < NG:
                gtiles[g + 2] = _issue_gather(g + 2)
            _expand_group(g + 1, _sum_group(g + 1, gtiles.pop(g + 1)))
        hn = [None, None]
        for gb in range(2):
            cg = 128 * g + 16 * ds + 8 * gb  # ET/eW cols for this step+group
            bks = slice(HB * gb, HB * (gb + 1))
            hg = h[gb]

            # --- gate path first: its ScalarE exp must not queue behind
            # the relu, and its PE matmuls must finish before the U ones.
            pm = psS.tile([128, BK], F32, tag=f"pm{gb}", name=f"pm{gb}")
            psg = pm[0:8, 0:HB]
            nc.tensor.matmul(psg, lhsT=ET[0][:, cg:cg + 8], rhs=kT[0][:, bks],
                             start=True, stop=False)
            nc.tensor.matmul(psg, lhsT=ET[1][:, cg:cg + 8], rhs=kT[1][:, bks],
                             start=False, stop=False)
            nc.tensor.matmul(psg, lhsT=ET[0][:, cg:cg + 8], rhs=hg[:, 0:HB],
                             start=False, stop=False)
            nc.tensor.matmul(psg, lhsT=ET[1][:, cg:cg + 8], rhs=hg[:, HB:BK],
                             start=False, stop=True)
            # eg = exp(-logits + mask_bias); masked rows get +45 so the
            # sigmoid underflows to 0 and h carries through unchanged.
            eg = work.tile([8, HB], F32, tag=f"eg{gb}", name=f"eg{gb}")
            gm = work.tile([8, HB], BF16, tag=f"gm{gb}", name=f"gm{gb}")
            nc.scalar.activation(eg[:], psg, AF.Exp, scale=-1.0,
                                 bias=sb_mb[0:8, 2 * t + gb:2 * t + gb + 1])
            # gate = 1/(eg + bd): bd=1 on-diag, 1e12 off-diag -> gate ~ 0
            nc.vector._custom_dve(_RECIP1P_BD, out=gm[:], in0=eg[:],
                                  in1=bd8[:],
                                  s0=float(_R1P_C0), s1=float(_R1P_C1),
                                  imm2=float(_R1P_C2))

            # --- h_tilda pre-activation: kVT + eW + U^T h (h terms last)
            pshG = psH.tile([128, BK], F32, tag=f"psh{gb}", name=f"psh{gb}")
            for m in range(2):
                msl = slice(HB * m, HB * (m + 1))
                nc.tensor.matmul(pshG[:, msl], lhsT=I128[:], rhs=kVT[m][:, bks],
                                 start=True, stop=False)
                ew_bc = eW[m][:, cg:cg + 8].unsqueeze(2).broadcast_to([128, 8, 32])
                nc.tensor.matmul(pshG[:, msl], lhsT=I128[:], rhs=ew_bc,
                                 start=False, stop=False)
                nc.tensor.matmul(pshG[:, msl], lhsT=sbU[0][:, 128 * m:128 * (m + 1)],
                                 rhs=hg[:, 0:HB], start=False, stop=False)
                nc.tensor.matmul(pshG[:, msl], lhsT=sbU[1][:, 128 * m:128 * (m + 1)],
                                 rhs=hg[:, HB:BK], start=False, stop=True)

            # gate broadcast (both column halves)
            psbg = psS.tile([128, BK], F32, tag=f"psbg{gb}", name=f"psbg{gb}")
            nc.tensor.matmul(psbg[:, 0:HB], lhsT=ones8[:], rhs=gm[:],
                             start=True, stop=True)
            nc.tensor.matmul(psbg[:, HB:BK], lhsT=ones8[:], rhs=gm[:],
                             start=True, stop=True)

            # r = relu(psh); u = r*gate; upd = u + h
            r = work.tile([128, BK], BF16, tag=f"r{gb}", name=f"r{gb}")
            nc.scalar.activation(r[:], pshG[:], AF.Relu)
            u = work.tile([128, BK], BF16, tag=f"u{gb}", name=f"u{gb}")
            nc.vector.tensor_tensor(out=u[:], in0=r[:], in1=psbg[:, 0:BK], op=OP.mult)
            upd = work.tile([128, BK], BF16, tag=f"upd{gb}", name=f"upd{gb}")
            nc.vector.tensor_tensor(out=upd[:], in0=u[:], in1=hg[:], op=OP.add)

            # sumsq: fused d-half square-add then one 128-partition reduce
            sqh = work.tile([128, HB], BF16, tag=f"sq{gb}", name=f"sq{gb}")
            nc.vector._custom_dve(_SQADD, out=sqh[:], in0=upd[:, 0:HB],
                                  in1=upd[:, HB:BK])
            pss = pm[0:1, HB:BK]
            lns = work.tile([1, HB], F32, tag=f"lns{gb}", name=f"lns{gb}")
            inv = work.tile([1, HB], BF16, tag=f"inv{gb}", name=f"inv{gb}")
            pnv = psS.tile([128, BK], F32, tag=f"pnv{gb}", name=f"pnv{gb}")
            hn[gb] = hpool.tile([128, BK], BF16, tag=f"h{gb}", name=f"hn{gb}")
            nc.tensor.matmul(pss, lhsT=ones128[:], rhs=sqh[:],
                             start=True, stop=True)
            nc.scalar.activation(lns[:], pss, AF.Ln, bias=epsap[:])
            nc.scalar.activation(inv[:], lns[:], AF.Exp, scale=-0.5)
            nc.tensor.matmul(pnv[:, 0:HB], lhsT=ones1[:], rhs=inv[:],
                             start=True, stop=True)
            nc.tensor.matmul(pnv[:, HB:BK], lhsT=ones1[:], rhs=inv[:],
                             start=True, stop=True)
            nc.vector.tensor_tensor(out=hn[gb][:], in0=upd[:],
                                    in1=pnv[:, 0:BK], op=OP.mult)
        h = hn

    # ---- output: dump the transposed state; the host untransposes ----
    for gb in range(2):
        nc.sync.dma_start(out=hout[128 * gb:128 * (gb + 1), :], in_=h[gb][:])

    ctx.close()


def _prep_core(pr, mask, keys_c, emb):
    """Host-side marshaling for one core's shard."""
    uniq, inv = np.unique(pr, return_inverse=True)
    assert len(uniq) <= TABLE_ROWS
    table = np.zeros((TABLE_ROWS, D), dtype=ml_dtypes.bfloat16)
    table[: len(uniq)] = emb[uniq].astype(ml_dtypes.bfloat16)
    ranks = inv.reshape(BL, S, L).astype(np.int16)

    # token order per group g: i = (ds*16 + b)*32 + w
    idx_groups = []
    for g in range(NG):
        blk = ranks[:, 8 * g:8 * (g + 1), :]          # [b, ds, w]
        lst = blk.transpose(1, 0, 2).reshape(-1)      # [(ds, b, w)] length 4096
        idx_groups.append(np.tile(lst.reshape(TOKG // 16, 16).T, (8, 1)))
    idx16 = np.concatenate(idx_groups, axis=1).astype(np.int16)  # [128, NG*256]

    keysT = np.ascontiguousarray(
        keys_c.reshape(BK, D).T).astype(ml_dtypes.bfloat16)      # [256, 512]
    # mb[j, 2t+gb] = MASK_BIAS * (1 - mask[8*gb + j, t])
    m = mask.astype(np.float32)                                  # [16, 64]
    mb = np.zeros((8, 2 * S), np.float32)
    for gb in range(2):
        mb[:, gb::2] = MASK_BIAS * (1.0 - m[8 * gb:8 * (gb + 1), :])
    return table, idx16, keysT, mb


def kernel(prgrph, prgrph_mask, keys, embedding_matrix, U, V, W):
    prgrph = np.asarray(prgrph)
    prgrph_mask = np.asarray(prgrph_mask)
    keys = np.asarray(keys, dtype=np.float32)
    emb = np.asarray(embedding_matrix, dtype=np.float32)
    U = np.asarray(U, dtype=np.float32)
    V = np.asarray(V, dtype=np.float32)
    W = np.asarray(W, dtype=np.float32)

    if "nc" not in _CACHED:
        _CACHED["nc"] = _build_program()
    nc = _CACHED["nc"]

    Ub, Vb, Wb = (x.astype(ml_dtypes.bfloat16) for x in (U, V, W))
    bdc = np.where(np.arange(8)[:, None] == (np.arange(256)[None, :] // K),
                   1.0, 1e12).astype(ml_dtypes.bfloat16)

    in_maps = []
    for c in range(NC):
        sl = slice(BL * c, BL * (c + 1))
        table, idx16, keysT, mb = _prep_core(
            prgrph[sl], prgrph_mask[sl, :, 0], keys[sl], emb)
        in_maps.append({
            "table": table, "idx16": idx16, "keysT": keysT,
            "Umat": Ub, "Vmat": Vb, "Wmat": Wb,
            "mbias": mb, "bdin": bdc,
        })

    res = run_bass_kernel_spmd(nc, in_maps, core_ids=list(range(NC)))
    outs = []
    for c in range(NC):
        ht = np.asarray(res.results[c]["hout"], dtype=np.float32)  # [256, 512]
        # rows: [gb*128 + p] = d half m at partition p; cols: gb-local bk.
        # h^T[d, bk_global]: d = m*128 + p, bk_global = gb*256 + bk_local
        full = np.zeros((D, BK), np.float32)
        for gb in range(2):
            blk = ht[128 * gb:128 * (gb + 1), :]      # [128, 512] = (p, (m, bk))
            full[0:128, 256 * gb:256 * (gb + 1)] = blk[:, 0:256]
            full[128:256, 256 * gb:256 * (gb + 1)] = blk[:, 256:512]
        outs.append(full.T.reshape(BL, K, D))
    out = np.concatenate(outs, axis=0)
    return out.astype(np.float32)


# revision 46
# speedup vs baseline: 1.0139x; 1.0137x over previous
"""Trainium2 Bass kernel for nn_BasicRecurrentEntityEncoder.

Full-input contract: kernel(**inputs) takes the complete (unsharded) numpy
inputs and returns the full [B, K, D] float32 output. Internally the batch
is sharded over 8 NeuronCores (data parallel, no collectives), the embedding
bag-of-words gather runs through dma_gather against a per-core compacted
bf16 table, and the 64-step entity recurrence runs in a transposed
[D, (b,k)] layout with bf16 matmul operands.

Key device-side structure per core (B_local=16, K=32, D=256, S=64):
  - 8 gather groups of 128 sentences (4096 tokens, 1 dma_gather each);
    word-sum via block-ones matmuls into PSUM; TensorE transpose to build
    E^T [256, 1024] incrementally.
  - precompute  kVT = V^T keys^T  and  eW = W^T E^T  once per group.
  - the scan runs as TWO independent batch groups (b 0-7 | b 8-15), each
    with its own PSUM banks, so their serial dependency chains pipeline
    across engines.
  - per-step chain is latency-optimized: the gate path is issued FIRST
    (psg matmuls before the U matmuls so the ScalarE exp is not stuck
    behind the relu), the sentence mask is folded into the exp's
    per-partition bias (masked row -> +40 -> exp huge -> gate ~ 0, exact
    up to ~2e-6 because h is always 0 or unit-norm), the sigmoid recip
    and the block-diag junk-kill run as ONE custom DVE op (the "+1" of
    the sigmoid denominator is 1e12 on off-diagonal columns), and the
    two d-halves of the sum-of-squares run as ONE custom DVE op feeding
    a single 128-partition reduce matmul.
  - gather-group processing is interleaved into the scan emission
    (group g+1 is digested at step 8g+5) so the shared PSUM-bank tag
    slots alternate gather/scan; the word-sum burst runs half at demoted
    priority (fills PE idle gaps) and half at normal priority (ends the
    inv-bank WAR sooner).
  - the final state is DMAed out transposed; the host untransposes.
  - ScalarE: every function (Exp/Relu/Ln) lives in the
    natural_log_exp_and_others activation table so no table reloads
    occur (the default greedy chooser is patched out).
"""

import sys

if "/opt/trn_rl_repo" not in sys.path:
    sys.path.insert(0, "/opt/trn_rl_repo")

import numpy as np
import ml_dtypes

from concourse import bacc, mybir
import concourse.bass as bass
import concourse.tile as tile
from concourse.bass_utils import run_bass_kernel_spmd
from concourse.masks import make_identity

# Force every ScalarE activation onto the one table set that covers all the
# functions this kernel uses (relu/exp/ln/copy/identity). The default
# chooser greedily picks the first set per function, inserting a ~1283ns
# table reload per pair on the critical path. Padding the dict keeps
# act_func_set_id indices aligned with act_info.json while making only the
# all-covering set usable.
_ONE_SET = "natural_log_exp_and_others"


import concourse.hw_specs as _hw_specs
_ORIG_TABLES = _hw_specs.get_activation_tables


def _patched_tables(module_arch):
    real = _ORIG_TABLES(module_arch)
    names = list(real.keys())
    assert _ONE_SET in names, names
    out = {}
    for n in names:
        if n == _ONE_SET:
            out[n] = real[n]
            break
        out[n] = set()
    return out


def _install_table_patch():
    import functools
    cached = functools.cache(_patched_tables)
    bacc.get_activation_tables = cached
    _hw_specs.get_activation_tables = cached


_install_table_patch()

# ---------------------------------------------------------------------------
# Custom DVE ops.
#
# RECIP1P_BD: out ~= 1/(in0 + in1) in ONE VectorE instruction:
#   u = in0+in1; seed y0 = bitcast(~bits(u)); t = u*y0 lands in [-4.5, -4]
#   for any positive u; quadratic minimax fixup P(t) ~= 1/t gives
#   1/u = y0*P(t) at ~1e-5 relative error. in1 plays the "+1" of the
#   sigmoid on diagonal (own-batch) columns and is 1e12 on off-diagonal
#   columns, so junk logits produce a gate <= 1e-12 and vanish in the
#   row-sum broadcast matmul. The DVE pipeline has exactly 8 ALU blocks;
#   this uses all 8 (a separate mask multiply would not fit).
#
# SQADD: out = in0^2 + in1^2 — fuses the two d-halves of the per-column
#   sum-of-squares so one 128-partition reduce matmul finishes the norm.
# ---------------------------------------------------------------------------
import concourse.dve_ops as _dve_ops
from concourse.dve_spec import AluOp as _AluOp, Bin as _Bin, Spec as _Spec
from concourse.dve_spec import C0 as _C0, C1 as _C1, C2 as _C2, One as _One
from concourse.dve_spec import Src0 as _Src0, Src1 as _Src1, lower as _dve_lower
from concourse.dve_spec import _has_src1 as _dve_has_src1
from concourse.dve_uop import DveOpSpec as _DveOpSpec

_R1P_C2, _R1P_C1, _R1P_C0 = (lambda c: (c[0], c[1], c[2]))(
    np.polyfit(np.linspace(-4.5, -4.0, 2001),
               1.0 / np.linspace(-4.5, -4.0, 2001), 2))


def _recip1p_bd_ref(in0, in1, c0, c1, c2):
    u = (np.asarray(in0, np.float32) + np.asarray(in1, np.float32)).astype(
        np.float32)
    y0 = (~u.view(np.int32)).view(np.float32)
    t = u * y0
    return y0 * (c0 + t * (c1 + c2 * t))


def _sqadd_ref(in0, in1, c0, c1, c2):
    a = np.asarray(in0, np.float32)
    b = np.asarray(in1, np.float32)
    return a * a + b * b


def _register_op(name, spec):
    row = 1 + len(_dve_ops.OPS)
    assert row < 0x20
    shas = {}
    for ver in ("v3", "v4"):
        s = _DveOpSpec(name=name, opcode=row, uops=_dve_lower(spec, ver=ver),
                       rd1_en=_dve_has_src1(spec))
        shas[ver] = s.sha(ver)
    op = _dve_ops.DveOp(name, spec, subdim=False, uops_sha=shas)
    _dve_ops.OPS.append(op)
    _dve_ops._SUB_OPCODE_FOR_NAME[name] = row
    _dve_ops.CUSTOM_DVE_SPECS[name] = spec
    return op


def _make_recip1p_bd():
    u = _Bin(_AluOp.ADD, _Src0, _Src1)
    y0 = _Bin(_AluOp.BITWISE_NOT, u, u)
    t = u * y0
    body = y0 * (_C0 + t * (_C1 + _C2 * t))
    return _register_op("RECIP1P_BD_ANT",
                        _Spec(body=body, reference=_recip1p_bd_ref))


def _make_sqadd():
    body = _Src0 * _Src0 + _Src1 * _Src1
    return _register_op("SQADD_ANT", _Spec(body=body, reference=_sqadd_ref))


_RECIP1P_BD = _make_recip1p_bd()
_SQADD = _make_sqadd()

F32 = mybir.dt.float32
BF16 = mybir.dt.bfloat16
I16 = mybir.dt.int16
AF = mybir.ActivationFunctionType
OP = mybir.AluOpType

B, S, L, K, D = 128, 64, 32, 32, 256
NC = 8
BL = B // NC              # 16 batch rows per core
BK = BL * K               # 512 = free dim of the state
NG = 8                    # gather groups per core (128 sentences each)
TOKG = 128 * L            # 4096 tokens per group
TABLE_ROWS = 32768        # compacted per-core vocab (unique ids <= 32768)
EPS = 1e-12
MASK_BIAS = 40.0          # masked sentence: exp(-logit + 40) -> gate ~ 0

_CACHED = {}


def _build_program():
    nc = bacc.Bacc("TRN2", target_bir_lowering=False, debug=False, num_devices=NC)

    table = nc.dram_tensor("table", [TABLE_ROWS, D], BF16, kind="ExternalInput").ap()
    idx16 = nc.dram_tensor("idx16", [128, NG * TOKG // 16], I16, kind="ExternalInput").ap()
    keysT = nc.dram_tensor("keysT", [D, BK], BF16, kind="ExternalInput").ap()
    Umat = nc.dram_tensor("Umat", [D, D], BF16, kind="ExternalInput").ap()
    Vmat = nc.dram_tensor("Vmat", [D, D], BF16, kind="ExternalInput").ap()
    Wmat = nc.dram_tensor("Wmat", [D, D], BF16, kind="ExternalInput").ap()
    mbias = nc.dram_tensor("mbias", [8, 2 * S], F32, kind="ExternalInput").ap()
    bdin = nc.dram_tensor("bdin", [8, 256], BF16, kind="ExternalInput").ap()
    hout = nc.dram_tensor("hout", [2 * 128, BK], BF16, kind="ExternalOutput").ap()

    with tile.TileContext(nc) as tc:
        _emit(nc, tc, table, idx16, keysT, Umat, Vmat, Wmat, mbias, bdin, hout)
    nc.compile()
    return nc


def _emit(nc, tc, table, idx16, keysT, Umat, Vmat, Wmat, mbias, bdin, hout):
    from contextlib import ExitStack

    ctx = ExitStack()
    const = ctx.enter_context(tc.tile_pool(name="const", bufs=1))
    persist = ctx.enter_context(tc.tile_pool(name="persist", bufs=1))
    gpool = ctx.enter_context(tc.tile_pool(name="g", bufs=3))
    work = ctx.enter_context(tc.tile_pool(name="work", bufs=4))
    hpool = ctx.enter_context(tc.tile_pool(name="h", bufs=3))
    # PSUM budget: 8 banks. psh{0,1} = 2; psbg{0,1} (gate broadcast) = 2;
    # pnv{0,1} (inv broadcast) = 2; pm{0,1} (psg logits + pss sumsq; also
    # reused by the gather phase) = 2. The inv broadcast must NOT share the
    # pm bank: the next step's gate matmuls would inherit a WAR dependency
    # on hn's read of it, putting them back on the critical chain.
    psH = ctx.enter_context(tc.tile_pool(name="psH", bufs=1, space="PSUM"))
    psS = ctx.enter_context(tc.tile_pool(name="psS", bufs=1, space="PSUM"))

    # ---- constants into SBUF ----
    sb_idx = const.tile([128, NG * TOKG // 16], I16)
    nc.sync.dma_start(out=sb_idx[:, 0:TOKG // 16], in_=idx16[:, 0:TOKG // 16])
    nc.sync.dma_start(out=sb_idx[:, TOKG // 16:], in_=idx16[:, TOKG // 16:])
    kT = [const.tile([128, BK], BF16, tag=f"kT{j}", name=f"kT{j}") for j in range(2)]
    for j in range(2):
        nc.sync.dma_start(out=kT[j][:], in_=keysT[128 * j:128 * (j + 1), :])
    sbU = [const.tile([128, D], BF16, tag=f"sbU{j}", name=f"sbU{j}") for j in range(2)]
    sbV = [const.tile([128, D], BF16, tag=f"sbV{j}", name=f"sbV{j}") for j in range(2)]
    sbW = [const.tile([128, D], BF16, tag=f"sbW{j}", name=f"sbW{j}") for j in range(2)]
    for j in range(2):
        nc.sync.dma_start(out=sbU[j][:], in_=Umat[128 * j:128 * (j + 1), :])
        nc.sync.dma_start(out=sbV[j][:], in_=Vmat[128 * j:128 * (j + 1), :])
        nc.sync.dma_start(out=sbW[j][:], in_=Wmat[128 * j:128 * (j + 1), :])
    sb_mb = const.tile([8, 2 * S], F32)
    nc.sync.dma_start(out=sb_mb[:], in_=mbias[:])

    I128 = const.tile([128, 128], BF16)
    make_identity(nc, I128[:])
    ones8 = const.tile([8, 128], BF16)
    nc.vector.memset(ones8[:], 1.0)
    ones128 = const.tile([128, 1], BF16)
    nc.vector.memset(ones128[:], 1.0)
    ones1 = const.tile([1, 128], BF16)
    nc.vector.memset(ones1[:], 1.0)
    epsap = const.tile([1, 1], F32)
    nc.vector.memset(epsap[:], EPS)
    # sigmoid denominator offset: 1.0 on own-batch (diagonal) columns, 1e12
    # elsewhere so junk logits produce a ~0 gate (see RECIP1P_BD above).
    bd8 = const.tile([8, 256], BF16)
    nc.sync.dma_start(out=bd8[:], in_=bdin[:])
    # word-sum reducers: Ablk[i][p, m] = 1 iff m == 4*i + p//32.
    Ablk = []
    for i in range(16):
        a = const.tile([128, 64], BF16, tag=f"Ablk{i}", name=f"Ablk{i}")
        nc.vector.memset(a[:], 0.0)
        for q in range(4):
            nc.vector.memset(a[32 * q:32 * (q + 1), 4 * i + q:4 * i + q + 1], 1.0)
        Ablk.append(a)

    # ---- persistent intermediates ----
    ET = [persist.tile([128, NG * 128], BF16, tag=f"ET{j}", name=f"ET{j}") for j in range(2)]   # E^T  [d, (g,ds,b)]
    eW = [persist.tile([128, NG * 128], BF16, tag=f"eWt{j}", name=f"eWt{j}") for j in range(2)]   # W^T E^T
    kVT = [persist.tile([128, BK], BF16, tag=f"kVT{j}", name=f"kVT{j}") for j in range(2)]        # V^T keys^T

    # kVT = V^T @ keysT   (out[de, bk] = sum_d V[d,de] keysT[d,bk])
    for m in range(2):
        ps = psH.tile([128, BK], F32, tag="psh1", name="pskv")
        nc.tensor.matmul(ps[:], lhsT=sbV[0][:, 128 * m:128 * (m + 1)], rhs=kT[0][:],
                         start=True, stop=False)
        nc.tensor.matmul(ps[:], lhsT=sbV[1][:, 128 * m:128 * (m + 1)], rhs=kT[1][:],
                         start=False, stop=True)
        nc.vector.tensor_copy(out=kVT[m][:], in_=ps[:])

    # ---- gather machinery ----
    # Group processing is interleaved into the scan emission (group g+1 is
    # digested at step 8g+4) so the shared PSUM-bank tag slots alternate
    # gather/scan naturally; emitting all gathers up front would serialize
    # scan step 0 behind every group's word-sum through the shared banks.
    def _issue_gather(g):
        G = gpool.tile([128, L, D], BF16, tag="G", name=f"G{g}")
        nc.gpsimd.dma_gather(
            out_ap=G[:], in_ap=table[:],
            idxs_ap=sb_idx[:, (TOKG // 16) * g:(TOKG // 16) * (g + 1)],
            num_idxs=TOKG, num_idxs_reg=TOKG, elem_size=D, single_packet=False,
        )
        return G

    def _sum_group(g, G):
        # word-sum: slot c holds words of sentences 4c..4c+3; accumulate
        # 8 slots per 32-aligned PSUM block. psE shares the pnv0 scan bank
        # (longest per-step idle window); the burst runs at demoted
        # priority so simultaneously-ready scan ops win the engine heaps.
        psE = psS.tile([128, D], F32, tag="pnv0", name="psE")
        enc = work.tile([128, D], BF16, tag="enc", name=f"enc{g}")
        with tc.high_priority(offset=-10**6):
            for c in range(L // 2):
                j, i = c // 16, c % 16
                nc.tensor.matmul(psE[64 * j:64 * (j + 1), :], lhsT=Ablk[i][:],
                                 rhs=G[:, c, :], start=(i == 0), stop=(i == 15))
        # second half at normal priority: ends the burst (and the bank WAR)
        # sooner at the cost of briefly preempting scan matmuls
        for c in range(L // 2, L):
            j, i = c // 16, c % 16
            nc.tensor.matmul(psE[64 * j:64 * (j + 1), :], lhsT=Ablk[i][:],
                             rhs=G[:, c, :], start=(i == 0), stop=(i == 15))
        nc.scalar.copy(out=enc[:], in_=psE[:])
        return enc

    def _expand_group(g, enc):
        with tc.high_priority(offset=-10**6):
            # transpose -> ET columns for this group
            for j in range(2):
                pt = psS.tile([128, 128], BF16, tag="pm1", name="pt")
                nc.tensor.transpose(pt[:], enc[:, 128 * j:128 * (j + 1)], I128[:])
                nc.vector.tensor_copy(out=ET[j][:, 128 * g:128 * (g + 1)],
                                      in_=pt[:])
            # eW = W^T @ ET_g
            for m in range(2):
                pw = psS.tile([128, 128], F32, tag="pm1", name="pw")
                nc.tensor.matmul(pw[:], lhsT=sbW[0][:, 128 * m:128 * (m + 1)],
                                 rhs=ET[0][:, 128 * g:128 * (g + 1)],
                                 start=True, stop=False)
                nc.tensor.matmul(pw[:], lhsT=sbW[1][:, 128 * m:128 * (m + 1)],
                                 rhs=ET[1][:, 128 * g:128 * (g + 1)],
                                 start=False, stop=True)
                nc.vector.tensor_copy(out=eW[m][:, 128 * g:128 * (g + 1)],
                                      in_=pw[:])

    # Group 0 is gathered and digested in two 64-sentence halves: step 0
    # only needs the first 16 ET columns, so the scan starts after half
    # the gather DMA and half the word-sum. Half b is digested at t==1
    # (its psE bank slot then sits AFTER pnv0(t=0), where there is slack).
    def _issue_half_gather(half):
        Gh = gpool.tile([128, L // 2, D], BF16, tag="G", name=f"G0{half}")
        nc.gpsimd.dma_gather(
            out_ap=Gh[:], in_ap=table[:],
            idxs_ap=sb_idx[:, 128 * half:128 * (half + 1)],
            num_idxs=TOKG // 2, num_idxs_reg=TOKG // 2, elem_size=D,
            single_packet=False,
        )
        return Gh

    def _half_process(half, Gh):
        psEh = psS.tile([64, D], F32, tag="pnv0", name=f"psE0{half}")
        ench = work.tile([64, D], BF16, tag=f"ench{half}", name=f"ench{half}")
        for i in range(16):
            nc.tensor.matmul(psEh[0:64, :], lhsT=Ablk[i][:], rhs=Gh[:, i, :],
                             start=(i == 0), stop=(i == 15))
        nc.scalar.copy(out=ench[:], in_=psEh[0:64, :])
        for j in range(2):
            pt = psS.tile([128, 64], BF16, tag="pm1", name="pt")
            nc.tensor.matmul(pt[:], lhsT=ench[:, 128 * j:128 * (j + 1)],
                             rhs=I128[0:64, 0:64], is_transpose=True,
                             start=True, stop=True)
            nc.vector.tensor_copy(out=ET[j][:, 64 * half:64 * (half + 1)],
                                  in_=pt[:])
        for m in range(2):
            pw = psS.tile([128, 64], F32, tag="pm1", name="pw")
            nc.tensor.matmul(pw[:], lhsT=sbW[0][:, 128 * m:128 * (m + 1)],
                             rhs=ET[0][:, 64 * half:64 * (half + 1)],
                             start=True, stop=False)
            nc.tensor.matmul(pw[:], lhsT=sbW[1][:, 128 * m:128 * (m + 1)],
                             rhs=ET[1][:, 64 * half:64 * (half + 1)],
                             start=False, stop=True)
            nc.vector.tensor_copy(out=eW[m][:, 64 * half:64 * (half + 1)],
                                  in_=pw[:])

    gtiles = {}
    gencs = {}
    G0a = _issue_half_gather(0)
    G0b = _issue_half_gather(1)
    gtiles[1] = _issue_gather(1)
    _half_process(0, G0a)

    # ---- scan: two independent batch groups (b 0-7 | b 8-15) pipelined ----
    # State h[gb] [128, 512]: partitions = d mod 128, columns pack
    # (d-half, bk) for the group's 256 bk entries.
    HB = BK // 2  # 256
    h = [hpool.tile([128, BK], BF16, tag=f"h{gb}", name=f"h{gb}")
         for gb in range(2)]
    for gb in range(2):
        nc.vector.memset(h[gb][:], 0.0)

    for t in range(S):
        g, ds = t // 8, t % 8
        if t == 1:
            with tc.high_priority(offset=-10**6):
                _half_process(1, G0b)
        elif ds == 5 and g + 1 < NG:
            if g + 2 < NG:
                gtiles[g + 2] = _issue_gather(g + 2)
            _expand_group(g + 1, _sum_group(g + 1, gtiles.pop(g + 1)))
        hn = [None, None]
        for gb in range(2):
            cg = 128 * g + 16 * ds + 8 * gb  # ET/eW cols for this step+group
            bks = slice(HB * gb, HB * (gb + 1))
            hg = h[gb]

            # --- gate path first: its ScalarE exp must not queue behind
            # the relu, and its PE matmuls must finish before the U ones.
            pm = psS.tile([128, BK], F32, tag=f"pm{gb}", name=f"pm{gb}")
            psg = pm[0:8, 0:HB]
            nc.tensor.matmul(psg, lhsT=ET[0][:, cg:cg + 8], rhs=kT[0][:, bks],
                             start=True, stop=False)
            nc.tensor.matmul(psg, lhsT=ET[1][:, cg:cg + 8], rhs=kT[1][:, bks],
                             start=False, stop=False)
            nc.tensor.matmul(psg, lhsT=ET[0][:, cg:cg + 8], rhs=hg[:, 0:HB],
                             start=False, stop=False)
            nc.tensor.matmul(psg, lhsT=ET[1][:, cg:cg + 8], rhs=hg[:, HB:BK],
                             start=False, stop=True)
            # eg = exp(-logits + mask_bias); masked rows get +45 so the
            # sigmoid underflows to 0 and h carries through unchanged.
            eg = work.tile([8, HB], F32, tag=f"eg{gb}", name=f"eg{gb}")
            gm = work.tile([8, HB], BF16, tag=f"gm{gb}", name=f"gm{gb}")
            nc.scalar.activation(eg[:], psg, AF.Exp, scale=-1.0,
                                 bias=sb_mb[0:8, 2 * t + gb:2 * t + gb + 1])
            # gate = 1/(eg + bd): bd=1 on-diag, 1e12 off-diag -> gate ~ 0
            nc.vector._custom_dve(_RECIP1P_BD, out=gm[:], in0=eg[:],
                                  in1=bd8[:],
                                  s0=float(_R1P_C0), s1=float(_R1P_C1),
                                  imm2=float(_R1P_C2))

            # --- h_tilda pre-activation: kVT + eW + U^T h (h terms last)
            pshG = psH.tile([128, BK], F32, tag=f"psh{gb}", name=f"psh{gb}")
            for m in range(2):
                msl = slice(HB * m, HB * (m + 1))
                nc.tensor.matmul(pshG[:, msl], lhsT=I128[:], rhs=kVT[m][:, bks],
                                 start=True, stop=False)
                ew_bc = eW[m][:, cg:cg + 8].unsqueeze(2).broadcast_to([128, 8, 32])
                nc.tensor.matmul(pshG[:, msl], lhsT=I128[:], rhs=ew_bc,
                                 start=False, stop=False)
                nc.tensor.matmul(pshG[:, msl], lhsT=sbU[0][:, 128 * m:128 * (m + 1)],
                                 rhs=hg[:, 0:HB], start=False, stop=False)
                nc.tensor.matmul(pshG[:, msl], lhsT=sbU[1][:, 128 * m:128 * (m + 1)],
                                 rhs=hg[:, HB:BK], start=False, stop=True)

            # gate broadcast (both column halves)
            psbg = psS.tile([128, BK], F32, tag=f"psbg{gb}", name=f"psbg{gb}")
            nc.tensor.matmul(psbg[:, 0:HB], lhsT=ones8[:], rhs=gm[:],
                             start=True, stop=True)
            nc.tensor.matmul(psbg[:, HB:BK], lhsT=ones8[:], rhs=gm[:],
                             start=True, stop=True)

            # r = relu(psh); u = r*gate; upd = u + h
            r = work.tile([128, BK], BF16, tag=f"r{gb}", name=f"r{gb}")
            nc.scalar.activation(r[:], pshG[:], AF.Relu)
            u = work.tile([128, BK], BF16, tag=f"u{gb}", name=f"u{gb}")
            nc.vector.tensor_tensor(out=u[:], in0=r[:], in1=psbg[:, 0:BK], op=OP.mult)
            upd = work.tile([128, BK], BF16, tag=f"upd{gb}", name=f"upd{gb}")
            nc.vector.tensor_tensor(out=upd[:], in0=u[:], in1=hg[:], op=OP.add)

            # sumsq: fused d-half square-add then one 128-partition reduce
            sqh = work.tile([128, HB], BF16, tag=f"sq{gb}", name=f"sq{gb}")
            nc.vector._custom_dve(_SQADD, out=sqh[:], in0=upd[:, 0:HB],
                                  in1=upd[:, HB:BK])
            pss = pm[0:1, HB:BK]
            lns = work.tile([1, HB], F32, tag=f"lns{gb}", name=f"lns{gb}")
            inv = work.tile([1, HB], BF16, tag=f"inv{gb}", name=f"inv{gb}")
            pnv = psS.tile([128, BK], F32, tag=f"pnv{gb}", name=f"pnv{gb}")
            hn[gb] = hpool.tile([128, BK], BF16, tag=f"h{gb}", name=f"hn{gb}")
            nc.tensor.matmul(pss, lhsT=ones128[:], rhs=sqh[:],
                             start=True, stop=True)
            nc.scalar.activation(lns[:], pss, AF.Ln, bias=epsap[:])
            nc.scalar.activation(inv[:], lns[:], AF.Exp, scale=-0.5)
            nc.tensor.matmul(pnv[:, 0:HB], lhsT=ones1[:], rhs=inv[:],
                             start=True, stop=True)
            nc.tensor.matmul(pnv[:, HB:BK], lhsT=ones1[:], rhs=inv[:],
                             start=True, stop=True)
            nc.vector.tensor_tensor(out=hn[gb][:], in0=upd[:],
                                    in1=pnv[:, 0:BK], op=OP.mult)
        h = hn

    # ---- output: dump the transposed state; the host untransposes ----
    for gb in range(2):
        nc.sync.dma_start(out=hout[128 * gb:128 * (gb + 1), :], in_=h[gb][:])

    ctx.close()


def _prep_core(pr, mask, keys_c, emb):
    """Host-side marshaling for one core's shard."""
    uniq, inv = np.unique(pr, return_inverse=True)
    assert len(uniq) <= TABLE_ROWS
    table = np.zeros((TABLE_ROWS, D), dtype=ml_dtypes.bfloat16)
    table[: len(uniq)] = emb[uniq].astype(ml_dtypes.bfloat16)
    ranks = inv.reshape(BL, S, L).astype(np.int16)

    # token order per group g: i = (ds*16 + b)*32 + w
    idx_groups = []
    for g in range(NG):
        blk = ranks[:, 8 * g:8 * (g + 1), :]          # [b, ds, w]
        lst = blk.transpose(1, 0, 2).reshape(-1)      # [(ds, b, w)] length 4096
        idx_groups.append(np.tile(lst.reshape(TOKG // 16, 16).T, (8, 1)))
    idx16 = np.concatenate(idx_groups, axis=1).astype(np.int16)  # [128, NG*256]

    keysT = np.ascontiguousarray(
        keys_c.reshape(BK, D).T).astype(ml_dtypes.bfloat16)      # [256, 512]
    # mb[j, 2t+gb] = MASK_BIAS * (1 - mask[8*gb + j, t])
    m = mask.astype(np.float32)                                  # [16, 64]
    mb = np.zeros((8, 2 * S), np.float32)
    for gb in range(2):
        mb[:, gb::2] = MASK_BIAS * (1.0 - m[8 * gb:8 * (gb + 1), :])
    return table, idx16, keysT, mb


def kernel(prgrph, prgrph_mask, keys, embedding_matrix, U, V, W):
    prgrph = np.asarray(prgrph)
    prgrph_mask = np.asarray(prgrph_mask)
    keys = np.asarray(keys, dtype=np.float32)
    emb = np.asarray(embedding_matrix, dtype=np.float32)
    U = np.asarray(U, dtype=np.float32)
    V = np.asarray(V, dtype=np.float32)
    W = np.asarray(W, dtype=np.float32)

    if "nc" not in _CACHED:
        _CACHED["nc"] = _build_program()
    nc = _CACHED["nc"]

    Ub, Vb, Wb = (x.astype(ml_dtypes.bfloat16) for x in (U, V, W))
    bdc = np.where(np.arange(8)[:, None] == (np.arange(256)[None, :] // K),
                   1.0, 1e12).astype(ml_dtypes.bfloat16)

    in_maps = []
    for c in range(NC):
        sl = slice(BL * c, BL * (c + 1))
        table, idx16, keysT, mb = _prep_core(
            prgrph[sl], prgrph_mask[sl, :, 0], keys[sl], emb)
        in_maps.append({
            "table": table, "idx16": idx16, "keysT": keysT,
            "Umat": Ub, "Vmat": Vb, "Wmat": Wb,
            "mbias": mb, "bdin": bdc,
        })

    res = run_bass_kernel_spmd(nc, in_maps, core_ids=list(range(NC)))
    outs = []
    for c in range(NC):
        ht = np.asarray(res.results[c]["hout"], dtype=np.float32)  # [256, 512]
        # rows: [gb*128 + p] = d half m at partition p; cols: gb-local bk.
        # h^T[d, bk_global]: d = m*128 + p, bk_global = gb*256 + bk_local
        full = np.zeros((D, BK), np.float32)
        for gb in range(2):
            blk = ht[128 * gb:128 * (gb + 1), :]      # [128, 512] = (p, (m, bk))
            full[0:128, 256 * gb:256 * (gb + 1)] = blk[:, 0:256]
            full[128:256, 256 * gb:256 * (gb + 1)] = blk[:, 256:512]
        outs.append(full.T.reshape(BL, K, D))
    out = np.concatenate(outs, axis=0)
    return out.astype(np.float32)


# revision 50
# speedup vs baseline: 1.0141x; 1.0002x over previous
"""Trainium2 Bass kernel for nn_BasicRecurrentEntityEncoder.

Full-input contract: kernel(**inputs) takes the complete (unsharded) numpy
inputs and returns the full [B, K, D] float32 output. Internally the batch
is sharded over 8 NeuronCores (data parallel, no collectives), the embedding
bag-of-words gather runs through dma_gather against a per-core compacted
bf16 table, and the 64-step entity recurrence runs in a transposed
[D, (b,k)] layout with bf16 matmul operands.

Key device-side structure per core (B_local=16, K=32, D=256, S=64):
  - 8 gather groups of 128 sentences (4096 tokens, 1 dma_gather each);
    word-sum via block-ones matmuls into PSUM; TensorE transpose to build
    E^T [256, 1024] incrementally.
  - precompute  kVT = V^T keys^T  and  eW = W^T E^T  once per group.
  - the scan runs as TWO independent batch groups (b 0-7 | b 8-15), each
    with its own PSUM banks, so their serial dependency chains pipeline
    across engines.
  - per-step chain is latency-optimized: the gate path is issued FIRST
    (psg matmuls before the U matmuls so the ScalarE exp is not stuck
    behind the relu), the sentence mask is folded into the exp's
    per-partition bias (masked row -> +40 -> exp huge -> gate ~ 0, exact
    up to ~2e-6 because h is always 0 or unit-norm), the sigmoid recip
    and the block-diag junk-kill run as ONE custom DVE op (the "+1" of
    the sigmoid denominator is 1e12 on off-diagonal columns), and the
    two d-halves of the sum-of-squares run as ONE custom DVE op feeding
    a single 128-partition reduce matmul.
  - gather-group processing is interleaved into the scan emission
    (group g+1 is digested at step 8g+5) so the shared PSUM-bank tag
    slots alternate gather/scan; the word-sum burst runs half at demoted
    priority (fills PE idle gaps) and half at normal priority (ends the
    inv-bank WAR sooner). Group 0 is gathered and digested in two
    64-sentence halves so the scan starts after only half the first
    gather DMA; half b trickles in under step 1.
  - the final state is DMAed out transposed; the host untransposes.
  - ScalarE: every function (Exp/Relu/Ln) lives in the
    natural_log_exp_and_others activation table so no table reloads
    occur (the default greedy chooser is patched out).
"""

import sys

if "/opt/trn_rl_repo" not in sys.path:
    sys.path.insert(0, "/opt/trn_rl_repo")

import numpy as np
import ml_dtypes

from concourse import bacc, mybir
import concourse.bass as bass
import concourse.tile as tile
from concourse.bass_utils import run_bass_kernel_spmd
from concourse.masks import make_identity

# Force every ScalarE activation onto the one table set that covers all the
# functions this kernel uses (relu/exp/ln/copy/identity). The default
# chooser greedily picks the first set per function, inserting a ~1283ns
# table reload per pair on the critical path. Padding the dict keeps
# act_func_set_id indices aligned with act_info.json while making only the
# all-covering set usable.
_ONE_SET = "natural_log_exp_and_others"


import concourse.hw_specs as _hw_specs
_ORIG_TABLES = _hw_specs.get_activation_tables


def _patched_tables(module_arch):
    real = _ORIG_TABLES(module_arch)
    names = list(real.keys())
    assert _ONE_SET in names, names
    out = {}
    for n in names:
        if n == _ONE_SET:
            out[n] = real[n]
            break
        out[n] = set()
    return out


def _install_table_patch():
    import functools
    cached = functools.cache(_patched_tables)
    bacc.get_activation_tables = cached
    _hw_specs.get_activation_tables = cached


_install_table_patch()

# ---------------------------------------------------------------------------
# Custom DVE ops.
#
# RECIP1P_BD: out ~= 1/(in0 + in1) in ONE VectorE instruction:
#   u = in0+in1; seed y0 = bitcast(~bits(u)); t = u*y0 lands in [-4.5, -4]
#   for any positive u; quadratic minimax fixup P(t) ~= 1/t gives
#   1/u = y0*P(t) at ~1e-5 relative error. in1 plays the "+1" of the
#   sigmoid on diagonal (own-batch) columns and is 1e12 on off-diagonal
#   columns, so junk logits produce a gate <= 1e-12 and vanish in the
#   row-sum broadcast matmul. The DVE pipeline has exactly 8 ALU blocks;
#   this uses all 8 (a separate mask multiply would not fit).
#
# SQADD: out = in0^2 + in1^2 — fuses the two d-halves of the per-column
#   sum-of-squares so one 128-partition reduce matmul finishes the norm.
# ---------------------------------------------------------------------------
import concourse.dve_ops as _dve_ops
from concourse.dve_spec import AluOp as _AluOp, Bin as _Bin, Spec as _Spec
from concourse.dve_spec import C0 as _C0, C1 as _C1, C2 as _C2, One as _One
from concourse.dve_spec import Src0 as _Src0, Src1 as _Src1, lower as _dve_lower
from concourse.dve_spec import _has_src1 as _dve_has_src1
from concourse.dve_uop import DveOpSpec as _DveOpSpec

_R1P_C2, _R1P_C1, _R1P_C0 = (lambda c: (c[0], c[1], c[2]))(
    np.polyfit(np.linspace(-4.5, -4.0, 2001),
               1.0 / np.linspace(-4.5, -4.0, 2001), 2))


def _recip1p_bd_ref(in0, in1, c0, c1, c2):
    u = (np.asarray(in0, np.float32) + np.asarray(in1, np.float32)).astype(
        np.float32)
    y0 = (~u.view(np.int32)).view(np.float32)
    t = u * y0
    return y0 * (c0 + t * (c1 + c2 * t))


def _sqadd_ref(in0, in1, c0, c1, c2):
    a = np.asarray(in0, np.float32)
    b = np.asarray(in1, np.float32)
    return a * a + b * b


def _register_op(name, spec):
    row = 1 + len(_dve_ops.OPS)
    assert row < 0x20
    shas = {}
    for ver in ("v3", "v4"):
        s = _DveOpSpec(name=name, opcode=row, uops=_dve_lower(spec, ver=ver),
                       rd1_en=_dve_has_src1(spec))
        shas[ver] = s.sha(ver)
    op = _dve_ops.DveOp(name, spec, subdim=False, uops_sha=shas)
    _dve_ops.OPS.append(op)
    _dve_ops._SUB_OPCODE_FOR_NAME[name] = row
    _dve_ops.CUSTOM_DVE_SPECS[name] = spec
    return op


def _make_recip1p_bd():
    u = _Bin(_AluOp.ADD, _Src0, _Src1)
    y0 = _Bin(_AluOp.BITWISE_NOT, u, u)
    t = u * y0
    body = y0 * (_C0 + t * (_C1 + _C2 * t))
    return _register_op("RECIP1P_BD_ANT",
                        _Spec(body=body, reference=_recip1p_bd_ref))


def _make_sqadd():
    body = _Src0 * _Src0 + _Src1 * _Src1
    return _register_op("SQADD_ANT", _Spec(body=body, reference=_sqadd_ref))


_RECIP1P_BD = _make_recip1p_bd()
_SQADD = _make_sqadd()

F32 = mybir.dt.float32
BF16 = mybir.dt.bfloat16
I16 = mybir.dt.int16
AF = mybir.ActivationFunctionType
OP = mybir.AluOpType

B, S, L, K, D = 128, 64, 32, 32, 256
NC = 8
BL = B // NC              # 16 batch rows per core
BK = BL * K               # 512 = free dim of the state
NG = 8                    # gather groups per core (128 sentences each)
TOKG = 128 * L            # 4096 tokens per group
TABLE_ROWS = 32768        # compacted per-core vocab (unique ids <= 32768)
EPS = 1e-12
MASK_BIAS = 40.0          # masked sentence: exp(-logit + 40) -> gate ~ 0

_CACHED = {}


def _build_program():
    nc = bacc.Bacc("TRN2", target_bir_lowering=False, debug=False, num_devices=NC)

    table = nc.dram_tensor("table", [TABLE_ROWS, D], BF16, kind="ExternalInput").ap()
    idx16 = nc.dram_tensor("idx16", [128, NG * TOKG // 16], I16, kind="ExternalInput").ap()
    keysT = nc.dram_tensor("keysT", [D, BK], BF16, kind="ExternalInput").ap()
    Umat = nc.dram_tensor("Umat", [D, D], BF16, kind="ExternalInput").ap()
    Vmat = nc.dram_tensor("Vmat", [D, D], BF16, kind="ExternalInput").ap()
    Wmat = nc.dram_tensor("Wmat", [D, D], BF16, kind="ExternalInput").ap()
    mbias = nc.dram_tensor("mbias", [8, 2 * S], F32, kind="ExternalInput").ap()
    bdin = nc.dram_tensor("bdin", [8, 256], BF16, kind="ExternalInput").ap()
    hout = nc.dram_tensor("hout", [2 * 128, BK], BF16, kind="ExternalOutput").ap()

    with tile.TileContext(nc) as tc:
        _emit(nc, tc, table, idx16, keysT, Umat, Vmat, Wmat, mbias, bdin, hout)
    nc.compile()
    return nc


def _emit(nc, tc, table, idx16, keysT, Umat, Vmat, Wmat, mbias, bdin, hout):
    from contextlib import ExitStack

    ctx = ExitStack()
    const = ctx.enter_context(tc.tile_pool(name="const", bufs=1))
    persist = ctx.enter_context(tc.tile_pool(name="persist", bufs=1))
    gpool = ctx.enter_context(tc.tile_pool(name="g", bufs=4))
    work = ctx.enter_context(tc.tile_pool(name="work", bufs=4))
    hpool = ctx.enter_context(tc.tile_pool(name="h", bufs=3))
    # PSUM budget: 8 banks. psh{0,1} = 2; psbg{0,1} (gate broadcast) = 2;
    # pnv{0,1} (inv broadcast) = 2; pm{0,1} (psg logits + pss sumsq; also
    # reused by the gather phase) = 2. The inv broadcast must NOT share the
    # pm bank: the next step's gate matmuls would inherit a WAR dependency
    # on hn's read of it, putting them back on the critical chain.
    psH = ctx.enter_context(tc.tile_pool(name="psH", bufs=1, space="PSUM"))
    psS = ctx.enter_context(tc.tile_pool(name="psS", bufs=1, space="PSUM"))

    # ---- constants into SBUF ----
    sb_idx = const.tile([128, NG * TOKG // 16], I16)
    nc.sync.dma_start(out=sb_idx[:, 0:TOKG // 16], in_=idx16[:, 0:TOKG // 16])
    nc.sync.dma_start(out=sb_idx[:, TOKG // 16:], in_=idx16[:, TOKG // 16:])
    kT = [const.tile([128, BK], BF16, tag=f"kT{j}", name=f"kT{j}") for j in range(2)]
    for j in range(2):
        nc.sync.dma_start(out=kT[j][:], in_=keysT[128 * j:128 * (j + 1), :])
    sbU = [const.tile([128, D], BF16, tag=f"sbU{j}", name=f"sbU{j}") for j in range(2)]
    sbV = [const.tile([128, D], BF16, tag=f"sbV{j}", name=f"sbV{j}") for j in range(2)]
    sbW = [const.tile([128, D], BF16, tag=f"sbW{j}", name=f"sbW{j}") for j in range(2)]
    for j in range(2):
        nc.sync.dma_start(out=sbU[j][:], in_=Umat[128 * j:128 * (j + 1), :])
        nc.sync.dma_start(out=sbV[j][:], in_=Vmat[128 * j:128 * (j + 1), :])
        nc.sync.dma_start(out=sbW[j][:], in_=Wmat[128 * j:128 * (j + 1), :])
    sb_mb = const.tile([8, 2 * S], F32)
    nc.sync.dma_start(out=sb_mb[:], in_=mbias[:])

    I128 = const.tile([128, 128], BF16)
    make_identity(nc, I128[:])
    ones8 = const.tile([8, 128], BF16)
    nc.vector.memset(ones8[:], 1.0)
    ones128 = const.tile([128, 1], BF16)
    nc.vector.memset(ones128[:], 1.0)
    ones1 = const.tile([1, 128], BF16)
    nc.vector.memset(ones1[:], 1.0)
    epsap = const.tile([1, 1], F32)
    nc.vector.memset(epsap[:], EPS)
    # sigmoid denominator offset: 1.0 on own-batch (diagonal) columns, 1e12
    # elsewhere so junk logits produce a ~0 gate (see RECIP1P_BD above).
    bd8 = const.tile([8, 256], BF16)
    nc.sync.dma_start(out=bd8[:], in_=bdin[:])
    # word-sum reducers: Ablk[i][p, m] = 1 iff m == 4*i + p//32.
    Ablk = []
    for i in range(16):
        a = const.tile([128, 64], BF16, tag=f"Ablk{i}", name=f"Ablk{i}")
        nc.vector.memset(a[:], 0.0)
        for q in range(4):
            nc.vector.memset(a[32 * q:32 * (q + 1), 4 * i + q:4 * i + q + 1], 1.0)
        Ablk.append(a)

    # ---- persistent intermediates ----
    ET = [persist.tile([128, NG * 128], BF16, tag=f"ET{j}", name=f"ET{j}") for j in range(2)]   # E^T  [d, (g,ds,b)]
    eW = [persist.tile([128, NG * 128], BF16, tag=f"eWt{j}", name=f"eWt{j}") for j in range(2)]   # W^T E^T
    kVT = [persist.tile([128, BK], BF16, tag=f"kVT{j}", name=f"kVT{j}") for j in range(2)]        # V^T keys^T

    # kVT = V^T @ keysT   (out[de, bk] = sum_d V[d,de] keysT[d,bk])
    for m in range(2):
        ps = psH.tile([128, BK], F32, tag="psh1", name="pskv")
        nc.tensor.matmul(ps[:], lhsT=sbV[0][:, 128 * m:128 * (m + 1)], rhs=kT[0][:],
                         start=True, stop=False)
        nc.tensor.matmul(ps[:], lhsT=sbV[1][:, 128 * m:128 * (m + 1)], rhs=kT[1][:],
                         start=False, stop=True)
        nc.vector.tensor_copy(out=kVT[m][:], in_=ps[:])

    # ---- gather machinery ----
    # Group processing is interleaved into the scan emission (group g+1 is
    # digested at step 8g+4) so the shared PSUM-bank tag slots alternate
    # gather/scan naturally; emitting all gathers up front would serialize
    # scan step 0 behind every group's word-sum through the shared banks.
    def _issue_gather(g):
        G = gpool.tile([128, L, D], BF16, tag="G", name=f"G{g}")
        nc.gpsimd.dma_gather(
            out_ap=G[:], in_ap=table[:],
            idxs_ap=sb_idx[:, (TOKG // 16) * g:(TOKG // 16) * (g + 1)],
            num_idxs=TOKG, num_idxs_reg=TOKG, elem_size=D, single_packet=False,
        )
        return G

    def _sum_group(g, G):
        # word-sum: slot c holds words of sentences 4c..4c+3; accumulate
        # 8 slots per 32-aligned PSUM block. psE shares the pnv0 scan bank
        # (longest per-step idle window); the burst runs at demoted
        # priority so simultaneously-ready scan ops win the engine heaps.
        psE = psS.tile([128, D], F32, tag="pnv0", name="psE")
        enc = work.tile([128, D], BF16, tag="enc", name=f"enc{g}")
        with tc.high_priority(offset=-10**6):
            for c in range(L // 2):
                j, i = c // 16, c % 16
                nc.tensor.matmul(psE[64 * j:64 * (j + 1), :], lhsT=Ablk[i][:],
                                 rhs=G[:, c, :], start=(i == 0), stop=(i == 15))
        # second half at normal priority: ends the burst (and the bank WAR)
        # sooner at the cost of briefly preempting scan matmuls
        for c in range(L // 2, L):
            j, i = c // 16, c % 16
            nc.tensor.matmul(psE[64 * j:64 * (j + 1), :], lhsT=Ablk[i][:],
                             rhs=G[:, c, :], start=(i == 0), stop=(i == 15))
        nc.scalar.copy(out=enc[:], in_=psE[:])
        return enc

    def _expand_group(g, enc):
        with tc.high_priority(offset=-10**6):
            # transpose -> ET columns for this group
            for j in range(2):
                pt = psS.tile([128, 128], BF16, tag="pm1", name="pt")
                nc.tensor.transpose(pt[:], enc[:, 128 * j:128 * (j + 1)], I128[:])
                nc.vector.tensor_copy(out=ET[j][:, 128 * g:128 * (g + 1)],
                                      in_=pt[:])
            # eW = W^T @ ET_g
            for m in range(2):
                pw = psS.tile([128, 128], F32, tag="pm1", name="pw")
                nc.tensor.matmul(pw[:], lhsT=sbW[0][:, 128 * m:128 * (m + 1)],
                                 rhs=ET[0][:, 128 * g:128 * (g + 1)],
                                 start=True, stop=False)
                nc.tensor.matmul(pw[:], lhsT=sbW[1][:, 128 * m:128 * (m + 1)],
                                 rhs=ET[1][:, 128 * g:128 * (g + 1)],
                                 start=False, stop=True)
                nc.vector.tensor_copy(out=eW[m][:, 128 * g:128 * (g + 1)],
                                      in_=pw[:])

    # Group 0 is gathered and digested in two 64-sentence halves: step 0
    # only needs the first 16 ET columns, so the scan starts after half
    # the gather DMA and half the word-sum. Half b is digested at t==1
    # (its psE bank slot then sits AFTER pnv0(t=0), where there is slack).
    def _issue_half_gather(half):
        Gh = gpool.tile([128, L // 2, D], BF16, tag="G", name=f"G0{half}")
        nc.gpsimd.dma_gather(
            out_ap=Gh[:], in_ap=table[:],
            idxs_ap=sb_idx[:, 128 * half:128 * (half + 1)],
            num_idxs=TOKG // 2, num_idxs_reg=TOKG // 2, elem_size=D,
            single_packet=False,
        )
        return Gh

    def _half_process(half, Gh):
        psEh = psS.tile([64, D], F32, tag="pnv0", name=f"psE0{half}")
        ench = work.tile([64, D], BF16, tag=f"ench{half}", name=f"ench{half}")
        for i in range(16):
            nc.tensor.matmul(psEh[0:64, :], lhsT=Ablk[i][:], rhs=Gh[:, i, :],
                             start=(i == 0), stop=(i == 15))
        nc.scalar.copy(out=ench[:], in_=psEh[0:64, :])
        for j in range(2):
            pt = psS.tile([128, 64], BF16, tag="pm1", name="pt")
            nc.tensor.matmul(pt[:], lhsT=ench[:, 128 * j:128 * (j + 1)],
                             rhs=I128[0:64, 0:64], is_transpose=True,
                             start=True, stop=True)
            nc.vector.tensor_copy(out=ET[j][:, 64 * half:64 * (half + 1)],
                                  in_=pt[:])
        for m in range(2):
            pw = psS.tile([128, 64], F32, tag="pm1", name="pw")
            nc.tensor.matmul(pw[:], lhsT=sbW[0][:, 128 * m:128 * (m + 1)],
                             rhs=ET[0][:, 64 * half:64 * (half + 1)],
                             start=True, stop=False)
            nc.tensor.matmul(pw[:], lhsT=sbW[1][:, 128 * m:128 * (m + 1)],
                             rhs=ET[1][:, 64 * half:64 * (half + 1)],
                             start=False, stop=True)
            nc.vector.tensor_copy(out=eW[m][:, 64 * half:64 * (half + 1)],
                                  in_=pw[:])

    gtiles = {}
    gencs = {}
    G0a = _issue_half_gather(0)
    G0b = _issue_half_gather(1)
    gtiles[1] = _issue_gather(1)
    _half_process(0, G0a)

    # ---- scan: two independent batch groups (b 0-7 | b 8-15) pipelined ----
    # State h[gb] [128, 512]: partitions = d mod 128, columns pack
    # (d-half, bk) for the group's 256 bk entries.
    HB = BK // 2  # 256
    h = [hpool.tile([128, BK], BF16, tag=f"h{gb}", name=f"h{gb}")
         for gb in range(2)]
    for gb in range(2):
        nc.vector.memset(h[gb][:], 0.0)

    for t in range(S):
        g, ds = t // 8, t % 8
        if t == 2:
            with tc.high_priority(offset=-10**6):
                _half_process(1, G0b)
        elif ds == 5 and g + 1 < NG:
            if g + 2 < NG:
                gtiles[g + 2] = _issue_gather(g + 2)
            _expand_group(g + 1, _sum_group(g + 1, gtiles.pop(g + 1)))
        hn = [None, None]
        for gb in range(2):
            cg = 128 * g + 16 * ds + 8 * gb  # ET/eW cols for this step+group
            bks = slice(HB * gb, HB * (gb + 1))
            hg = h[gb]

            # --- gate path first: its ScalarE exp must not queue behind
            # the relu, and its PE matmuls must finish before the U ones.
            pm = psS.tile([128, BK], F32, tag=f"pm{gb}", name=f"pm{gb}")
            psg = pm[0:8, 0:HB]
            nc.tensor.matmul(psg, lhsT=ET[0][:, cg:cg + 8], rhs=kT[0][:, bks],
                             start=True, stop=False)
            nc.tensor.matmul(psg, lhsT=ET[1][:, cg:cg + 8], rhs=kT[1][:, bks],
                             start=False, stop=False)
            nc.tensor.matmul(psg, lhsT=ET[0][:, cg:cg + 8], rhs=hg[:, 0:HB],
                             start=False, stop=False)
            nc.tensor.matmul(psg, lhsT=ET[1][:, cg:cg + 8], rhs=hg[:, HB:BK],
                             start=False, stop=True)
            # eg = exp(-logits + mask_bias); masked rows get +45 so the
            # sigmoid underflows to 0 and h carries through unchanged.
            eg = work.tile([8, HB], F32, tag=f"eg{gb}", name=f"eg{gb}")
            gm = work.tile([8, HB], BF16, tag=f"gm{gb}", name=f"gm{gb}")
            nc.scalar.activation(eg[:], psg, AF.Exp, scale=-1.0,
                                 bias=sb_mb[0:8, 2 * t + gb:2 * t + gb + 1])
            # gate = 1/(eg + bd): bd=1 on-diag, 1e12 off-diag -> gate ~ 0
            nc.vector._custom_dve(_RECIP1P_BD, out=gm[:], in0=eg[:],
                                  in1=bd8[:],
                                  s0=float(_R1P_C0), s1=float(_R1P_C1),
                                  imm2=float(_R1P_C2))

            # --- h_tilda pre-activation: kVT + eW + U^T h (h terms last)
            pshG = psH.tile([128, BK], F32, tag=f"psh{gb}", name=f"psh{gb}")
            for m in range(2):
                msl = slice(HB * m, HB * (m + 1))
                nc.tensor.matmul(pshG[:, msl], lhsT=I128[:], rhs=kVT[m][:, bks],
                                 start=True, stop=False)
                ew_bc = eW[m][:, cg:cg + 8].unsqueeze(2).broadcast_to([128, 8, 32])
                nc.tensor.matmul(pshG[:, msl], lhsT=I128[:], rhs=ew_bc,
                                 start=False, stop=False)
                nc.tensor.matmul(pshG[:, msl], lhsT=sbU[0][:, 128 * m:128 * (m + 1)],
                                 rhs=hg[:, 0:HB], start=False, stop=False)
                nc.tensor.matmul(pshG[:, msl], lhsT=sbU[1][:, 128 * m:128 * (m + 1)],
                                 rhs=hg[:, HB:BK], start=False, stop=True)

            # gate broadcast (both column halves)
            psbg = psS.tile([128, BK], F32, tag=f"psbg{gb}", name=f"psbg{gb}")
            nc.tensor.matmul(psbg[:, 0:HB], lhsT=ones8[:], rhs=gm[:],
                             start=True, stop=True)
            nc.tensor.matmul(psbg[:, HB:BK], lhsT=ones8[:], rhs=gm[:],
                             start=True, stop=True)

            # r = relu(psh); u = r*gate; upd = u + h
            r = work.tile([128, BK], BF16, tag=f"r{gb}", name=f"r{gb}")
            nc.scalar.activation(r[:], pshG[:], AF.Relu)
            u = work.tile([128, BK], BF16, tag=f"u{gb}", name=f"u{gb}")
            nc.vector.tensor_tensor(out=u[:], in0=r[:], in1=psbg[:, 0:BK], op=OP.mult)
            upd = work.tile([128, BK], BF16, tag=f"upd{gb}", name=f"upd{gb}")
            nc.vector.tensor_tensor(out=upd[:], in0=u[:], in1=hg[:], op=OP.add)

            # sumsq: fused d-half square-add then one 128-partition reduce
            sqh = work.tile([128, HB], BF16, tag=f"sq{gb}", name=f"sq{gb}")
            nc.vector._custom_dve(_SQADD, out=sqh[:], in0=upd[:, 0:HB],
                                  in1=upd[:, HB:BK])
            pss = pm[0:1, HB:BK]
            lns = work.tile([1, HB], F32, tag=f"lns{gb}", name=f"lns{gb}")
            inv = work.tile([1, HB], BF16, tag=f"inv{gb}", name=f"inv{gb}")
            pnv = psS.tile([128, BK], F32, tag=f"pnv{gb}", name=f"pnv{gb}")
            hn[gb] = hpool.tile([128, BK], BF16, tag=f"h{gb}", name=f"hn{gb}")
            nc.tensor.matmul(pss, lhsT=ones128[:], rhs=sqh[:],
                             start=True, stop=True)
            nc.scalar.activation(lns[:], pss, AF.Ln, bias=epsap[:])
            nc.scalar.activation(inv[:], lns[:], AF.Exp, scale=-0.5)
            nc.tensor.matmul(pnv[:, 0:HB], lhsT=ones1[:], rhs=inv[:],
                             start=True, stop=True)
            nc.tensor.matmul(pnv[:, HB:BK], lhsT=ones1[:], rhs=inv[:],
                             start=True, stop=True)
            nc.vector.tensor_tensor(out=hn[gb][:], in0=upd[:],
                                    in1=pnv[:, 0:BK], op=OP.mult)
        h = hn

    # ---- output: dump the transposed state; the host untransposes ----
    for gb in range(2):
        nc.sync.dma_start(out=hout[128 * gb:128 * (gb + 1), :], in_=h[gb][:])

    ctx.close()


def _prep_core(pr, mask, keys_c, emb):
    """Host-side marshaling for one core's shard."""
    uniq, inv = np.unique(pr, return_inverse=True)
    assert len(uniq) <= TABLE_ROWS
    table = np.zeros((TABLE_ROWS, D), dtype=ml_dtypes.bfloat16)
    table[: len(uniq)] = emb[uniq].astype(ml_dtypes.bfloat16)
    ranks = inv.reshape(BL, S, L).astype(np.int16)

    # token order per group g: i = (ds*16 + b)*32 + w
    idx_groups = []
    for g in range(NG):
        blk = ranks[:, 8 * g:8 * (g + 1), :]          # [b, ds, w]
        lst = blk.transpose(1, 0, 2).reshape(-1)      # [(ds, b, w)] length 4096
        idx_groups.append(np.tile(lst.reshape(TOKG // 16, 16).T, (8, 1)))
    idx16 = np.concatenate(idx_groups, axis=1).astype(np.int16)  # [128, NG*256]

    keysT = np.ascontiguousarray(
        keys_c.reshape(BK, D).T).astype(ml_dtypes.bfloat16)      # [256, 512]
    # mb[j, 2t+gb] = MASK_BIAS * (1 - mask[8*gb + j, t])
    m = mask.astype(np.float32)                                  # [16, 64]
    mb = np.zeros((8, 2 * S), np.float32)
    for gb in range(2):
        mb[:, gb::2] = MASK_BIAS * (1.0 - m[8 * gb:8 * (gb + 1), :])
    return table, idx16, keysT, mb


def kernel(prgrph, prgrph_mask, keys, embedding_matrix, U, V, W):
    prgrph = np.asarray(prgrph)
    prgrph_mask = np.asarray(prgrph_mask)
    keys = np.asarray(keys, dtype=np.float32)
    emb = np.asarray(embedding_matrix, dtype=np.float32)
    U = np.asarray(U, dtype=np.float32)
    V = np.asarray(V, dtype=np.float32)
    W = np.asarray(W, dtype=np.float32)

    if "nc" not in _CACHED:
        _CACHED["nc"] = _build_program()
    nc = _CACHED["nc"]

    Ub, Vb, Wb = (x.astype(ml_dtypes.bfloat16) for x in (U, V, W))
    bdc = np.where(np.arange(8)[:, None] == (np.arange(256)[None, :] // K),
                   1.0, 1e12).astype(ml_dtypes.bfloat16)

    in_maps = []
    for c in range(NC):
        sl = slice(BL * c, BL * (c + 1))
        table, idx16, keysT, mb = _prep_core(
            prgrph[sl], prgrph_mask[sl, :, 0], keys[sl], emb)
        in_maps.append({
            "table": table, "idx16": idx16, "keysT": keysT,
            "Umat": Ub, "Vmat": Vb, "Wmat": Wb,
            "mbias": mb, "bdin": bdc,
        })

    res = run_bass_kernel_spmd(nc, in_maps, core_ids=list(range(NC)))
    outs = []
    for c in range(NC):
        ht = np.asarray(res.results[c]["hout"], dtype=np.float32)  # [256, 512]
        # rows: [gb*128 + p] = d half m at partition p; cols: gb-local bk.
        # h^T[d, bk_global]: d = m*128 + p, bk_global = gb*256 + bk_local
        full = np.zeros((D, BK), np.float32)
        for gb in range(2):
            blk = ht[128 * gb:128 * (gb + 1), :]      # [128, 512] = (p, (m, bk))
            full[0:128, 256 * gb:256 * (gb + 1)] = blk[:, 0:256]
            full[128:256, 256 * gb:256 * (gb + 1)] = blk[:, 256:512]
        outs.append(full.T.reshape(BL, K, D))
    out = np.concatenate(outs, axis=0)
    return out.astype(np.float32)
